# revision 2
# baseline (speedup 1.0000x reference)
"""DistVisionTransformer (STOSA-style ViT, mean+cov paths) on 8 Trainium2
NeuronCores. Data-parallel: one image per core; full forward pass on-device
in float32r (TF32-like) matmuls with fp32 accumulation.

Layout: activations are feature-major [768, 394] where columns 0:197 are the
mean-path tokens and 197:394 the cov-path tokens (cls token at cols 0 / 197).
LayerNorm / softmax partition-axis reductions are done with ones-vector
matmuls on the PE; per-token broadcasts with rank-1 ones outer products.
"""
import numpy as np
from contextlib import ExitStack

import concourse.bass as bass
import concourse.bacc as bacc
import concourse.tile as tile
import concourse.mybir as mybir
from concourse.bass_utils import run_bass_kernel_spmd
from concourse.masks import make_identity

F32 = mybir.dt.float32
F32R = mybir.dt.float32r
AF = mybir.ActivationFunctionType
ALU = mybir.AluOpType

B, E, H, L, P, IMG, NCLS = 8, 768, 12, 12, 16, 224, 1000
D = E // H                  # 64
MLP = 4 * E                 # 3072
SCALE = D ** -0.5
NPATCH = (IMG // P) ** 2    # 196
N = NPATCH + 1              # 197
T2 = 2 * N                  # 394  (mean | cov concatenated along tokens)
NP256 = 256                 # padded token free-dim for 256-wide matmuls
KT = E // 128               # 6 k-tiles over features
MT_H = MLP // 128           # 24 hidden tiles
COVW = T2 - NP256           # 138: start of the 256-wide cov window
COVO = N - COVW             # 59: offset of cov data inside that window

_CACHE = {}


# --------------------------------------------------------------------------
# device kernel builder
# --------------------------------------------------------------------------

def build_nc(debug=False, repeats=1):
    nc = bacc.Bacc(None, target_bir_lowering=False)
    lp = nc.allow_low_precision("tf32-style kernel; fp32 psum accumulation")
    lp.__enter__()

    dp = nc.declare_dram_parameter
    xcol_d = dp("xcol", [E, NP256], F32R, isOutput=False)        # per-core im2col
    qkvw_d = dp("qkvw", [L, E, 3 * E], F32R, isOutput=False)     # [in, out] (q|k|v)
    projw_d = dp("projw", [L, E, E], F32R, isOutput=False)
    cprojw_d = dp("cprojw", [L, E, E], F32R, isOutput=False)
    fc1w_d = dp("fc1w", [L, E, MLP], F32R, isOutput=False)
    fc2w_d = dp("fc2w", [L, MLP, E], F32R, isOutput=False)
    pw_d = dp("pw", [E, E], F32R, isOutput=False)                # patch embed [in, out]
    cpw_d = dp("cpw", [E, E], F32R, isOutput=False)
    headw_d = dp("headw", [E, 1024], F32R, isOutput=False)       # [in, out] padded
    rpbT_d = dp("rpbT", [H, N, NP256], F32, isOutput=False)      # rpb[h].T, padded
    acols_d = dp("acols", [L, 128, 36], F32, isOutput=False)     # per-tile param cols
    bq_d = dp("bq", [L, 128, 24], F32, isOutput=False)           # qkv psum bias cols
    fc1b_d = dp("fc1b", [L, 128, 24], F32, isOutput=False)
    vb_d = dp("vb", [L, 1, 2 * E], F32R, isOutput=False)         # v / cov_v bias rows
    pbrow_d = dp("pbrow", [L, 1, 3 * E], F32R, isOutput=False)   # proj|cproj|fc2 bias rows
    ones_d = dp("ones", [128, 520], F32R, isOutput=False)        # all-ones block
    maskneg_d = dp("maskneg", [E, H], F32R, isOutput=False)      # -1 blockdiag
    clspb_d = dp("clspb", [E, 4], F32R, isOutput=False)          # cls|cov_cls|patch_b|cov_patch_b
    fcn_d = dp("fcn", [E, 2], F32, isOutput=False)               # fc_norm g|b
    headb_d = dp("headb", [128, 8], F32, isOutput=False)
    zeros_d = dp("zeros", [1, H * N + 64], F32R, isOutput=False)
    out_d = dp("out", [1024, 1], F32, isOutput=True)
    if debug:
        dbg_d = dp("dbg", [L + 1, E, T2], F32R, isOutput=True)

    with tile.TileContext(nc) as tc, ExitStack() as ctx:
        pp = ctx.enter_context(tc.tile_pool(name="persist", bufs=1))
        wp = ctx.enter_context(tc.tile_pool(name="weights", bufs=14))
        rp = ctx.enter_context(tc.tile_pool(name="rows", bufs=1))
        bp = ctx.enter_context(tc.tile_pool(name="bigscratch", bufs=2))
        ap = ctx.enter_context(tc.tile_pool(name="attn", bufs=2))
        ps = ctx.enter_context(tc.tile_pool(name="psum", bufs=8, space="PSUM"))

        _bn = [0]

        def bank(shape):
            _bn[0] += 1
            return ps.tile(shape, F32, tag="bank", name=f"pb{_bn[0]}")

        _wn = [0]

        def wchunk():
            _wn[0] += 1
            return wp.tile([128, E], F32R, tag="wchunk", name=f"w{_wn[0]}")

        # ---- persistent constants ----
        ones = pp.tile([128, 520], F32R)       # columns / rows of ones
        nc.sync.dma_start(ones[:], ones_d[:])
        onescol = ones[:, 0:1]
        onesrow = ones[0:1, :]
        mask = [pp.tile([128, H], F32R, tag=f"mask{t}", name=f"mask{t}") for t in range(KT)]
        for t in range(KT):
            nc.sync.dma_start(mask[t][:], maskneg_d[128 * t:128 * (t + 1), :])
        clspb = [pp.tile([128, 4], F32R, tag=f"clspb{t}", name=f"clspb{t}") for t in range(KT)]
        for t in range(KT):
            nc.sync.dma_start(clspb[t][:], clspb_d[128 * t:128 * (t + 1), :])
        ident = pp.tile([128, 128], F32)
        make_identity(nc, ident[:])

        # ---- persistent state / per-layer reused buffers ----
        xs = [pp.tile([128, T2], F32R, tag=f"xs{t}", name=f"xs{t}") for t in range(KT)]
        xn = [pp.tile([128, T2], F32R, tag=f"xn{t}", name=f"xn{t}") for t in range(KT)]
        xsq = [pp.tile([128, T2], F32R, tag=f"xsq{t}", name=f"xsq{t}") for t in range(KT)]
        qkvs = [pp.tile([128, T2], F32R, tag=f"qkvs{t}", name=f"qkvs{t}") for t in range(2 * KT)]
        sqk = [pp.tile([128, NP256], F32R, tag=f"sqk{t}", name=f"sqk{t}") for t in range(2 * KT)]
        vtm = {}
        for path in (0, 1):
            vtm[path] = (pp.tile([128, E], F32R, tag=f"vtm{path}a", name=f"vtm{path}a"),
                         pp.tile([N - 128, E], F32R, tag=f"vtm{path}b", name=f"vtm{path}b"))
        ctx_s = [pp.tile([128, T2], F32R, tag=f"ctx{t}", name=f"ctx{t}") for t in range(KT)]
        gath = pp.tile([1, H * N + 64], F32R)
        nc.sync.dma_start(gath[:], zeros_d[:])   # zero the pad columns once
        an_t = pp.tile([H, N], F32R)
        bn_t = pp.tile([H, N], F32)
        bT = (pp.tile([128, H], F32, tag="bT0", name="bT0"), pp.tile([N - 128, H], F32, tag="bT1", name="bT1"))
        stage = pp.tile([64, T2], F32R)
        eps_t = pp.tile([1, 1], F32)
        nc.vector.memset(eps_t[:], 1e-5)

        MTOK = (128, N - 128)      # token m-tile sizes
        eps = 1e-5

        def layer_norm(src, g_ap, b_ap):
            """src: 6 [128,T2] f32r tiles -> xn (feature-major LN over partitions)."""
            for t in range(KT):
                nc.scalar.activation(xsq[t][:], src[t][:], AF.Square)
            p_s = bank([1, T2])
            p_s2 = bank([1, T2])
            for t in range(KT):
                nc.tensor.matmul(p_s[:], onescol, src[t][:],
                                 start=(t == 0), stop=(t == KT - 1))
            for t in range(KT):
                nc.tensor.matmul(p_s2[:], onescol, xsq[t][:],
                                 start=(t == 0), stop=(t == KT - 1))
            mu = rp.tile([1, T2], F32, tag="ln_mu")
            nc.vector.tensor_scalar(mu[:], p_s[:], 1.0 / E, None, ALU.mult)
            var = rp.tile([1, T2], F32, tag="ln_var")
            nc.vector.tensor_scalar(var[:], p_s2[:], 1.0 / E, None, ALU.mult)
            musq = rp.tile([1, T2], F32, tag="ln_musq")
            nc.scalar.activation(musq[:], mu[:], AF.Square)
            nc.vector.tensor_sub(var[:], var[:], musq[:])
            std = rp.tile([1, T2], F32, tag="ln_std")
            nc.scalar.activation(std[:], var[:], AF.Sqrt, bias=eps_t[:])
            rinv = rp.tile([1, T2], F32, tag="ln_rinv")
            nc.vector.reciprocal(rinv[:], std[:])
            rinv_r = rp.tile([1, T2], F32R, tag="ln_rinvr")
            nc.scalar.activation(rinv_r[:], rinv[:], AF.Copy)
            mur = rp.tile([1, T2], F32R, tag="ln_mur")
            nc.vector.tensor_mul(mur[:], mu[:], rinv_r[:])
            p_R = bank([128, T2])
            p_MR = bank([128, T2])
            nc.tensor.matmul(p_R[:], onesrow[:, 0:128], rinv_r[:], start=True, stop=True)
            nc.tensor.matmul(p_MR[:], onesrow[:, 0:128], mur[:], start=True, stop=True)
            for t in range(KT):
                tmp = bp.tile([128, T2], F32, tag="ln_tmp")
                nc.vector.tensor_mul(tmp[:], src[t][:], p_R[:])
                nc.vector.tensor_sub(tmp[:], tmp[:], p_MR[:])
                nc.scalar.activation(xn[t][:], tmp[:], AF.Identity,
                                     bias=b_ap(t), scale=g_ap(t))

        for _rep in range(repeats):
            # ================= patch embed =================
            xcol = [pp.tile([128, NP256], F32R, tag=f"xcol{t}", name=f"xcol{t}") for t in range(KT)]
            for t in range(KT):
                nc.sync.dma_start(xcol[t][:], xcol_d[128 * t:128 * (t + 1), :])

            for path, w_d in ((0, pw_d), (1, cpw_d)):
                wt = []
                for kt in range(KT):
                    w = wchunk()
                    nc.sync.dma_start(w[:], w_d[128 * kt:128 * (kt + 1), :])
                    wt.append(w)
                for mt in range(KT):
                    pe = bank([128, NP256])
                    for kt in range(KT):
                        nc.tensor.matmul(pe[:], wt[kt][:, 128 * mt:128 * (mt + 1)],
                                         xcol[kt][:], start=(kt == 0), stop=(kt == KT - 1))
                    dst = xs[mt][:, 1 + path * N: 1 + path * N + NPATCH]
                    nc.scalar.activation(dst, pe[:, 0:NPATCH], AF.Identity,
                                         bias=clspb[mt][:, 2 + path: 3 + path], scale=1.0)
            # cls tokens into cols 0 / 197
            for t in range(KT):
                nc.vector.tensor_copy(
                    xs[t][:, 0:T2].rearrange("p (a c) -> p a c", a=2)[:, :, 0:1],
                    clspb[t][:, 0:2].rearrange("p (a c) -> p a c", a=2)[:, :, 0:1])

            if debug:
                for t in range(KT):
                    nc.sync.dma_start(dbg_d[0, 128 * t:128 * (t + 1), :], xs[t][:])

            # ================= transformer layers =================
            for li in range(L):
                A = rp.tile([128, 36], F32, tag="acols")
                nc.sync.dma_start(A[:], acols_d[li])
                Bq = rp.tile([128, 24], F32, tag="bqcols")
                nc.sync.dma_start(Bq[:], bq_d[li])
                f1b = rp.tile([128, 24], F32, tag="fc1b")
                nc.sync.dma_start(f1b[:], fc1b_d[li])
                vb = rp.tile([1, 2 * E], F32R, tag="vbrow")
                nc.sync.dma_start(vb[:], vb_d[li])
                pbr = rp.tile([1, 3 * E], F32R, tag="pbrow")
                nc.sync.dma_start(pbr[:], pbrow_d[li])

                def ac(t, c):
                    return A[:, 6 * t + c: 6 * t + c + 1]

                # ---- LN1 ----
                layer_norm(xs, lambda t: ac(t, 0), lambda t: ac(t, 1))

                # ---- qkv (q|k part), feature-major, both paths at once ----
                qkw = {}
                for b in (0, 1):
                    for kt in range(KT):
                        w = wchunk()
                        nc.sync.dma_start(w[:], qkvw_d[li, 128 * kt:128 * (kt + 1),
                                                       E * b:E * (b + 1)])
                        qkw[(b, kt)] = w
                for mt in range(2 * KT):
                    b, m = mt // KT, mt % KT
                    pq = bank([128, T2])
                    for kt in range(KT):
                        nc.tensor.matmul(pq[:], qkw[(b, kt)][:, 128 * m:128 * (m + 1)],
                                         xn[kt][:], start=(kt == 0), stop=(kt == KT - 1))
                    # mean columns
                    if mt < KT:   # q rows: * 2*SCALE, + 2*SCALE*q_bias
                        nc.vector.tensor_scalar(qkvs[mt][:, 0:N], pq[:, 0:N],
                                                2.0 * SCALE, Bq[:, 2 * mt:2 * mt + 1],
                                                ALU.mult, ALU.add)
                    else:
                        nc.vector.tensor_copy(qkvs[mt][:, 0:N], pq[:, 0:N])
                    # cov columns: elu(x + b) + 1 = exp(min(x+b,0)) + max(x+b,0)
                    e1 = bp.tile([128, N], F32, tag="elu1")
                    e2 = bp.tile([128, N], F32, tag="elu2")
                    bcol = Bq[:, 2 * mt + 1:2 * mt + 2]
                    nc.vector.tensor_scalar(e1[:], pq[:, N:T2], bcol, 0.0, ALU.add, ALU.min)
                    nc.scalar.activation(e1[:], e1[:], AF.Exp)
                    nc.vector.tensor_scalar(e2[:], pq[:, N:T2], bcol, 0.0, ALU.add, ALU.max)
                    nc.vector.tensor_add(qkvs[mt][:, N:T2], e1[:], e2[:])

                # sq = 2*sqrt(cq) = sqrt(4 cq); sk = sqrt(ck)
                for mt in range(2 * KT):
                    nc.scalar.activation(sqk[mt][:, 0:N], qkvs[mt][:, N:T2], AF.Sqrt,
                                         scale=4.0 if mt < KT else 1.0)

                # ---- v token-major (both paths), weight as moving operand ----
                for kt in range(KT):
                    w = wchunk()
                    nc.sync.dma_start(w[:], qkvw_d[li, 128 * kt:128 * (kt + 1), 2 * E:3 * E])
                    qkw[(2, kt)] = w
                for path in (0, 1):
                    for Mt in range(2):
                        moff, mw = 128 * Mt, MTOK[Mt]
                        for ch in range(2):
                            pv = bank([mw, 384])
                            for kt in range(KT):
                                nc.tensor.matmul(
                                    pv[:], xn[kt][:, path * N + moff: path * N + moff + mw],
                                    qkw[(2, kt)][:, 384 * ch:384 * (ch + 1)],
                                    start=(kt == 0), stop=False)
                            nc.tensor.matmul(pv[:], onesrow[:, 0:mw],
                                             vb[:, path * E + 384 * ch: path * E + 384 * (ch + 1)],
                                             start=False, stop=True)
                            dst = vtm[path][Mt][:, 384 * ch:384 * (ch + 1)]
                            if path == 0:
                                nc.vector.tensor_copy(dst, pv[:])
                            else:
                                e1 = bp.tile([128, 384], F32, tag="velu1")
                                e2 = bp.tile([128, 384], F32, tag="velu2")
                                nc.vector.tensor_scalar(e1[0:mw, :], pv[:], 0.0, None, ALU.min)
                                nc.scalar.activation(e1[0:mw, :], e1[0:mw, :], AF.Exp)
                                nc.vector.tensor_scalar(e2[0:mw, :], pv[:], 0.0, None, ALU.max)
                                nc.vector.tensor_add(dst, e1[0:mw, :], e2[0:mw, :])

                # ---- a/b norm terms via -1-blockdiag mask matmuls ----
                p_a = bank([H, T2])
                p_b = bank([H, T2])
                for half, pdst in ((0, p_a), (1, p_b)):
                    for kt in range(KT):
                        src = qkvs[half * KT + kt]
                        nc.scalar.activation(xsq[kt][:, 0:N], src[:, 0:N], AF.Square,
                                             scale=0.5 if half == 0 else 1.0)
                        nc.vector.tensor_copy(xsq[kt][:, N:T2], src[:, N:T2])
                    for kt in range(KT):
                        nc.tensor.matmul(pdst[:], mask[kt][:], xsq[kt][:],
                                         start=(kt == 0), stop=(kt == KT - 1))
                nc.vector.tensor_copy(an_t[:], p_a[:, 0:N])
                nc.vector.tensor_add(an_t[:], an_t[:], p_a[:, N:T2])
                nc.vector.tensor_copy(bn_t[:], p_b[:, 0:N])
                nc.vector.tensor_add(bn_t[:], bn_t[:], p_b[:, N:T2])
                # gather -a rows to partition 0 (strided per-head 256 slots)
                nc.sync.dma_start(
                    gath[:, 0:H * N].rearrange("p (h c) -> p h c", c=N), an_t[:])
                # transpose -b to per-token columns
                for Mt in range(2):
                    moff, mw = 128 * Mt, MTOK[Mt]
                    pt = bank([mw, H])
                    nc.tensor.transpose(pt[:], bn_t[:, moff:moff + mw], ident[0:H, 0:H])
                    nc.vector.tensor_copy(bT[Mt][:], pt[:])

                # ---- attention, head by head ----
                for h in range(H):
                    qt, off = h // 2, 64 * (h % 2)
                    rpb_h = (ap.tile([128, NP256], F32, tag="rpba", name="rpba"),
                             ap.tile([N - 128, NP256], F32, tag="rpbb", name="rpbb"))
                    nc.sync.dma_start(rpb_h[0][:], rpbT_d[h, 0:128, :])
                    nc.sync.dma_start(rpb_h[1][:], rpbT_d[h, 128:N, :])
                    Et, E2t, psc = [], [], []
                    for Mt in range(2):
                        moff, mw = 128 * Mt, MTOK[Mt]
                        pc = bank([mw, NP256])
                        nc.tensor.matmul(pc[:], qkvs[KT + qt][off:off + 64, moff:moff + mw],
                                         qkvs[qt][off:off + 64, 0:NP256],
                                         start=True, stop=False)
                        nc.tensor.matmul(pc[:], sqk[KT + qt][off:off + 64, moff:moff + mw],
                                         sqk[qt][off:off + 64, 0:NP256],
                                         start=False, stop=False)
                        nc.tensor.matmul(pc[:], onesrow[:, 0:mw],
                                         gath[:, N * h:N * h + NP256],
                                         start=False, stop=True)
                        psc.append(pc)
                    for Mt in range(2):
                        mw = MTOK[Mt]
                        sg = ap.tile([128, NP256], F32, tag="sig")
                        nc.scalar.activation(sg[0:mw, :], psc[Mt][:], AF.Sigmoid,
                                             bias=bT[Mt][:, h:h + 1])
                        nc.vector.tensor_add(sg[0:mw, :], sg[0:mw, :], rpb_h[Mt][:])
                        Ee = ap.tile([128, NP256], F32R, tag="E")
                        nc.scalar.activation(Ee[0:mw, :], sg[0:mw, :], AF.Exp)
                        Et.append(Ee)
                    pd = bank([1, NP256])
                    for Mt in range(2):
                        mw = MTOK[Mt]
                        nc.tensor.matmul(pd[:], onescol[0:mw, :], Et[Mt][0:mw, :],
                                         start=(Mt == 0), stop=(Mt == 1))
                    rd = ap.tile([1, NP256], F32, tag="rd", bufs=2)
                    nc.vector.reciprocal(rd[:], pd[:])
                    rdr = ap.tile([1, NP256], F32R, tag="rdr", bufs=2)
                    nc.scalar.activation(rdr[:], rd[:], AF.Copy)
                    pr = bank([128, NP256])
                    nc.tensor.matmul(pr[:], onesrow[:, 0:128], rdr[:], start=True, stop=True)
                    for Mt in range(2):
                        mw = MTOK[Mt]
                        nc.vector.tensor_mul(Et[Mt][0:mw, :], Et[Mt][0:mw, :], pr[0:mw, :])
                        E2 = ap.tile([128, NP256], F32R, tag="E2")
                        nc.vector.tensor_mul(E2[0:mw, :], Et[Mt][0:mw, :], Et[Mt][0:mw, :])
                        E2t.append(E2)
                    pctx = bank([64, 512])
                    for path in (0, 1):
                        src = Et if path == 0 else E2t
                        for Mt in range(2):
                            mw = MTOK[Mt]
                            nc.tensor.matmul(pctx[:, 256 * path:256 * (path + 1)],
                                             vtm[path][Mt][:, 64 * h:64 * (h + 1)],
                                             src[Mt][0:mw, :],
                                             start=(Mt == 0), stop=(Mt == 1))
                    pv = pctx[:, 0:512].rearrange("p (a c) -> p a c", a=2)[:, :, 0:N]
                    if off == 0:
                        nc.vector.tensor_copy(
                            ctx_s[qt][0:64, 0:T2].rearrange("p (a c) -> p a c", a=2), pv)
                    else:
                        nc.vector.tensor_copy(
                            stage[:, 0:T2].rearrange("p (a c) -> p a c", a=2), pv)
                        nc.sync.dma_start(ctx_s[qt][64:128, :], stage[:])

                # ---- proj / cov_proj + gamma1-scaled residual ----
                for path, w_d in ((0, projw_d), (1, cprojw_d)):
                    pwt = []
                    for kt in range(KT):
                        w = wchunk()
                        nc.sync.dma_start(w[:], w_d[li, 128 * kt:128 * (kt + 1), :])
                        pwt.append(w)
                    win = 0 if path == 0 else COVW
                    vo = 0 if path == 0 else COVO
                    for mt in range(KT):
                        pj = bank([128, NP256])
                        for kt in range(KT):
                            nc.tensor.matmul(pj[:], pwt[kt][:, 128 * mt:128 * (mt + 1)],
                                             ctx_s[kt][:, win:win + NP256],
                                             start=(kt == 0), stop=False)
                        nc.tensor.matmul(pj[:], pbr[:, path * E + 128 * mt:
                                                    path * E + 128 * (mt + 1)],
                                         onesrow[:, 0:NP256], start=False, stop=True)
                        nc.vector.scalar_tensor_tensor(
                            xs[mt][:, path * N:(path + 1) * N], pj[:, vo:vo + N],
                            ac(mt, 4), xs[mt][:, path * N:(path + 1) * N],
                            ALU.mult, ALU.add)

                # ---- LN2 + MLP (chunked fc2 accumulation) ----
                layer_norm(xs, lambda t: ac(t, 2), lambda t: ac(t, 3))
                pf2 = [ps.tile([128, T2], F32, tag="bank", name=f"pf2_{_i}") for _i in range(KT)]
                f1w = {}

                def load_f1(jb):
                    for kt in range(KT):
                        w = wchunk()
                        nc.sync.dma_start(w[:], fc1w_d[li, 128 * kt:128 * (kt + 1),
                                                       E * jb:E * (jb + 1)])
                        f1w[(jb, kt)] = w

                load_f1(0)
                for j in range(MT_H):
                    jb, jm = j // KT, j % KT
                    if jm == 0 and jb + 1 < 4:
                        load_f1(jb + 1)
                    ph = bank([128, T2])
                    for kt in range(KT):
                        nc.tensor.matmul(ph[:], f1w[(jb, kt)][:, 128 * jm:128 * (jm + 1)],
                                         xn[kt][:], start=(kt == 0), stop=(kt == KT - 1))
                    Hj = bp.tile([128, T2], F32R, tag="hj")
                    nc.scalar.activation(Hj[:], ph[:], AF.Gelu, bias=f1b[:, j:j + 1])
                    w2 = wchunk()
                    nc.sync.dma_start(w2[:], fc2w_d[li, 128 * j:128 * (j + 1), :])
                    for i in range(KT):
                        nc.tensor.matmul(pf2[i][:], w2[:, 128 * i:128 * (i + 1)], Hj[:],
                                         start=(j == 0), stop=False,
                                         skip_group_check=True)
                for i in range(KT):
                    nc.tensor.matmul(pf2[i][:], pbr[:, 2 * E + 128 * i:2 * E + 128 * (i + 1)],
                                     onesrow[:, 0:T2], start=False, stop=True,
                                     skip_group_check=True)
                    nc.vector.scalar_tensor_tensor(xs[i][:], pf2[i][:], ac(i, 5), xs[i][:],
                                                   ALU.mult, ALU.add)

                if debug:
                    for t in range(KT):
                        nc.sync.dma_start(dbg_d[li + 1, 128 * t:128 * (t + 1), :], xs[t][:])

            # ================= head =================
            pl = [rp.tile([128, 2], F32R, tag=f"pool{t}", name=f"pool{t}") for t in range(KT)]
            for t in range(KT):
                nc.vector.tensor_reduce(pl[t][:, 0:1], xs[t][:, 1:N], mybir.AxisListType.X,
                                        ALU.add)
                nc.vector.tensor_scalar(pl[t][:, 0:1], pl[t][:, 0:1], 1.0 / NPATCH,
                                        None, ALU.mult)
                nc.scalar.activation(pl[t][:, 1:2], pl[t][:, 0:1], AF.Square)
            p_s = bank([1, 2])
            for t in range(KT):
                nc.tensor.matmul(p_s[:], onescol, pl[t][:],
                                 start=(t == 0), stop=(t == KT - 1))
            mu = rp.tile([1, 2], F32, tag="hmu")
            nc.vector.tensor_scalar(mu[:], p_s[:], 1.0 / E, None, ALU.mult)
            musq = rp.tile([1, 1], F32, tag="hmusq")
            nc.scalar.activation(musq[:], mu[:, 0:1], AF.Square)
            var = rp.tile([1, 1], F32, tag="hvar")
            nc.vector.tensor_sub(var[:], mu[:, 1:2], musq[:])
            std = rp.tile([1, 1], F32, tag="hstd")
            nc.scalar.activation(std[:], var[:], AF.Sqrt, bias=eps_t[:])
            rinv = rp.tile([1, 1], F32, tag="hrinv")
            nc.vector.reciprocal(rinv[:], std[:])
            rr = rp.tile([1, 2], F32R, tag="hrr")
            nc.scalar.activation(rr[:, 0:1], rinv[:], AF.Copy)
            nc.vector.tensor_mul(rr[:, 1:2], mu[:, 0:1], rr[:, 0:1])
            p_bc = bank([128, 2])
            nc.tensor.matmul(p_bc[:], onesrow[:, 0:128], rr[:], start=True, stop=True)
            fcn = [rp.tile([128, 2], F32, tag=f"fcn{t}", name=f"fcn{t}") for t in range(KT)]
            tn = [rp.tile([128, 2], F32R, tag=f"tn{t}", name=f"tn{t}") for t in range(KT)]
            for t in range(KT):
                nc.sync.dma_start(fcn[t][:], fcn_d[128 * t:128 * (t + 1), :])
                tmp = rp.tile([128, 1], F32, tag="htmp")
                nc.vector.tensor_mul(tmp[:], pl[t][:, 0:1], p_bc[:, 0:1])
                nc.vector.tensor_sub(tmp[:], tmp[:], p_bc[:, 1:2])
                nc.scalar.activation(tn[t][:, 0:1], tmp[:], AF.Identity,
                                     bias=fcn[t][:, 1:2], scale=fcn[t][:, 0:1])
                nc.vector.tensor_copy(tn[t][:, 1:2], tn[t][:, 0:1])
            hwt = {}
            for blk in range(2):
                for kt in range(KT):
                    w = wchunk()
                    wd = 768 if blk == 0 else 256
                    nc.sync.dma_start(w[:, 0:wd],
                                      headw_d[128 * kt:128 * (kt + 1),
                                              768 * blk:768 * blk + wd])
                    hwt[(blk, kt)] = w
            hb = rp.tile([128, 8], F32, tag="headb")
            nc.sync.dma_start(hb[:], headb_d[:])
            osb = rp.tile([128, 8], F32, tag="osb")
            for mt in range(8):
                blk, mo = (0, mt) if mt < 6 else (1, mt - 6)
                po = bank([128, 2])
                for kt in range(KT):
                    nc.tensor.matmul(po[:], hwt[(blk, kt)][:, 128 * mo:128 * (mo + 1)],
                                     tn[kt][:], start=(kt == 0), stop=(kt == KT - 1))
                nc.scalar.activation(osb[:, mt:mt + 1], po[:, 0:1], AF.Identity,
                                     bias=hb[:, mt:mt + 1])
            nc.sync.dma_start(out_d[:].rearrange("(a p) c -> p a c", p=128),
                              osb[:].rearrange("p (a c) -> p a c", c=1))

    lp.__exit__(None, None, None)
    nc.finalize()
    return nc


# --------------------------------------------------------------------------
# host-side input preparation
# --------------------------------------------------------------------------

def prep_shared(i):
    """Build the shared (weights etc.) input map from the full input dict."""
    f = np.float32

    def g(k):
        return np.asarray(i[k], dtype=f)

    qkvw = np.ascontiguousarray(np.transpose(g("qkv_w"), (0, 2, 1)))
    projw = np.ascontiguousarray(np.transpose(g("proj_w"), (0, 2, 1)))
    cprojw = np.ascontiguousarray(np.transpose(g("cov_proj_w"), (0, 2, 1)))
    fc1w = np.ascontiguousarray(np.transpose(g("fc1_w"), (0, 2, 1)))
    fc2w = np.ascontiguousarray(np.transpose(g("fc2_w"), (0, 2, 1)))
    pw = np.ascontiguousarray(g("patch_w").reshape(E, E).T)
    cpw = np.ascontiguousarray(g("cov_patch_w").reshape(E, E).T)
    headw = np.zeros((E, 1024), f)
    headw[:, 0:NCLS] = g("head_w").T
    rpbT = np.zeros((H, N, NP256), f)
    rpbT[:, :, 0:N] = np.transpose(g("rel_pos_bias"), (0, 2, 1))

    acols = np.zeros((L, 128, 36), f)
    for c, k in enumerate(["norm1_g", "norm1_b", "norm2_g", "norm2_b",
                           "gamma1", "gamma2"]):
        v = g(k).reshape(L, KT, 128)
        for t in range(KT):
            acols[:, :, 6 * t + c] = v[:, t, :]
    bq = np.zeros((L, 128, 24), f)
    qb2 = (2.0 * SCALE) * g("q_bias")
    cqb = g("cov_q_bias")
    for mt in range(KT):
        bq[:, :, 2 * mt] = qb2[:, 128 * mt:128 * (mt + 1)]
        bq[:, :, 2 * mt + 1] = cqb[:, 128 * mt:128 * (mt + 1)]
    fc1b = np.ascontiguousarray(g("fc1_b").reshape(L, 24, 128).transpose(0, 2, 1))
    vb = np.concatenate([g("v_bias"), g("cov_v_bias")], axis=1)[:, None, :]
    pbrow = np.concatenate([g("proj_b"), g("cov_proj_b"), g("fc2_b")],
                           axis=1)[:, None, :]
    ones = np.ones((128, 520), f)
    maskneg = np.zeros((E, H), f)
    for h in range(H):
        maskneg[64 * h:64 * (h + 1), h] = -1.0
    clspb = np.zeros((E, 4), f)
    clspb[:, 0] = g("cls_tok").reshape(E)
    clspb[:, 1] = g("cov_cls_tok").reshape(E)
    clspb[:, 2] = g("patch_b")
    clspb[:, 3] = g("cov_patch_b")
    fcn = np.stack([g("fc_norm_g"), g("fc_norm_b")], axis=1)
    hbp = np.zeros(1024, f)
    hbp[0:NCLS] = g("head_b")
    headb = np.ascontiguousarray(hbp.reshape(8, 128).T)
    zeros = np.zeros((1, H * N + 64), f)
    return {
        "qkvw": qkvw, "projw": projw, "cprojw": cprojw, "fc1w": fc1w,
        "fc2w": fc2w, "pw": pw, "cpw": cpw, "headw": headw, "rpbT": rpbT,
        "acols": acols, "bq": bq, "fc1b": fc1b, "vb": vb, "pbrow": pbrow,
        "ones": ones, "maskneg": maskneg, "clspb": clspb, "fcn": fcn,
        "headb": headb, "zeros": zeros,
    }


def im2col(x):
    """x: [B,3,224,224] -> [B, 768, 256] (zero-padded cols)."""
    f = np.float32
    xc = np.asarray(x, dtype=f).reshape(B, 3, 14, 16, 14, 16)
    xc = xc.transpose(0, 1, 3, 5, 2, 4).reshape(B, E, NPATCH)
    out = np.zeros((B, E, NP256), f)
    out[:, :, 0:NPATCH] = xc
    return out


def _get_nc(debug=False, repeats=1):
    key = ("nc", debug, repeats)
    if key not in _CACHE:
        _CACHE[key] = build_nc(debug=debug, repeats=repeats)
    return _CACHE[key]


def run(inputs, debug=False, trace=False, repeats=1, tmpdir=None):
    nc = _get_nc(debug=debug, repeats=repeats)
    shared = prep_shared(inputs)
    xcols = im2col(inputs["x"])
    in_maps = [dict(shared, xcol=np.ascontiguousarray(xcols[b])) for b in range(B)]
    res = run_bass_kernel_spmd(nc, in_maps, list(range(B)), trace=trace,
                               tmpdir=tmpdir)
    y = np.stack([res.results[b]["out"][0:NCLS, 0] for b in range(B)], axis=0)
    return y.astype(np.float32), res


def kernel(**inputs) -> np.ndarray:
    y, _ = run(inputs)
    return y



# revision 23
# speedup vs baseline: 1.1035x; 1.1035x over previous
"""DistVisionTransformer (STOSA-style ViT, mean+cov paths) on 8 Trainium2
NeuronCores. Data-parallel: one image per core; full forward pass on-device
in float32r (TF32-like) matmuls with fp32 accumulation.

Layout: activations are feature-major [768, 394] where columns 0:197 are the
mean-path tokens and 197:394 the cov-path tokens (cls token at cols 0 / 197).
LayerNorm / softmax partition-axis reductions are done with ones-vector
matmuls on the PE; per-token broadcasts with rank-1 ones outer products.
"""
import numpy as np
from contextlib import ExitStack

import concourse.bass as bass
import concourse.bacc as bacc
import concourse.tile as tile
import concourse.mybir as mybir
from concourse.bass_utils import run_bass_kernel_spmd
from concourse.masks import make_identity

F32 = mybir.dt.float32
F32R = mybir.dt.float32r
BF = mybir.dt.bfloat16
AF = mybir.ActivationFunctionType
ALU = mybir.AluOpType

B, E, H, L, P, IMG, NCLS = 8, 768, 12, 12, 16, 224, 1000
D = E // H                  # 64
MLP = 4 * E                 # 3072
SCALE = D ** -0.5
NPATCH = (IMG // P) ** 2    # 196
N = NPATCH + 1              # 197
T2 = 2 * N                  # 394  (mean | cov concatenated along tokens)
NP256 = 256                 # padded token free-dim for 256-wide matmuls
KT = E // 128               # 6 k-tiles over features
MT_H = MLP // 128           # 24 hidden tiles
COVW = T2 - NP256           # 138: start of the 256-wide cov window
COVO = N - COVW             # 59: offset of cov data inside that window

_CACHE = {}


# --------------------------------------------------------------------------
# device kernel builder
# --------------------------------------------------------------------------

def build_nc(debug=False, repeats=1):
    nc = bacc.Bacc(None, target_bir_lowering=False)
    lp = nc.allow_low_precision("tf32-style kernel; fp32 psum accumulation")
    lp.__enter__()

    dp = nc.declare_dram_parameter
    xcol_d = dp("xcol", [E, NP256], BF, isOutput=False)          # per-core im2col
    qkvw_d = dp("qkvw", [L, E, 3 * E], BF, isOutput=False)       # [in, out] (q|k|v)
    projw_d = dp("projw", [L, E, E], BF, isOutput=False)
    cprojw_d = dp("cprojw", [L, E, E], BF, isOutput=False)
    fc1w_d = dp("fc1w", [L, E, MLP], BF, isOutput=False)
    fc2w_d = dp("fc2w", [L, MLP, E], BF, isOutput=False)
    pw_d = dp("pw", [E, E], BF, isOutput=False)                  # patch embed [in, out]
    cpw_d = dp("cpw", [E, E], BF, isOutput=False)
    headw_d = dp("headw", [E, 1024], BF, isOutput=False)         # [in, out] padded
    rpbT_d = dp("rpbT", [H, N, NP256], BF, isOutput=False)       # rpb[h].T, padded
    acols_d = dp("acols", [L, 128, 36], F32, isOutput=False)     # per-tile param cols
    bq_d = dp("bq", [L, 128, 24], F32, isOutput=False)           # qkv psum bias cols
    fc1b_d = dp("fc1b", [L, 128, 24], F32, isOutput=False)
    vb_d = dp("vb", [L, 1, 2 * E], BF, isOutput=False)           # v / cov_v bias rows
    pbrow_d = dp("pbrow", [L, 1, 3 * E], BF, isOutput=False)     # proj|cproj|fc2 bias rows
    ones_d = dp("ones", [128, 520], F32R, isOutput=False)        # all-ones block
    onesb_d = dp("onesb", [128, 520], BF, isOutput=False)        # all-ones block bf16
    zeros_d = dp("zeros", [1, H * N + 64], BF, isOutput=False)
    maskneg_d = dp("maskneg", [E, H], BF, isOutput=False)        # -1 blockdiag
    clspb_d = dp("clspb", [E, 4], F32R, isOutput=False)          # cls|cov_cls|patch_b|cov_patch_b
    fcn_d = dp("fcn", [E, 2], F32, isOutput=False)               # fc_norm g|b
    headb_d = dp("headb", [128, 8], F32, isOutput=False)
    out_d = dp("out", [1024, 1], F32, isOutput=True)
    if debug:
        dbg_d = dp("dbg", [L + 1, E, T2], F32R, isOutput=True)

    with tile.TileContext(nc) as tc, ExitStack() as ctx:
        pp = ctx.enter_context(tc.tile_pool(name="persist", bufs=1))
        wp = ctx.enter_context(tc.tile_pool(name="weights", bufs=14))
        rp = ctx.enter_context(tc.tile_pool(name="rows", bufs=1))
        bp = ctx.enter_context(tc.tile_pool(name="bigscratch", bufs=2))
        ap = ctx.enter_context(tc.tile_pool(name="attn", bufs=2))
        ps = ctx.enter_context(tc.tile_pool(name="psum", bufs=8, space="PSUM"))

        _bn = [0]

        def bank(shape):
            _bn[0] += 1
            return ps.tile(shape, F32, tag="bank", name=f"pb{_bn[0]}")

        _wn = [0]

        def wchunk():
            _wn[0] += 1
            return wp.tile([128, E], BF, tag="wchunk", name=f"w{_wn[0]}")

        # ---- persistent constants ----
        ones = pp.tile([128, 520], F32R)       # columns / rows of ones (f32r)
        nc.sync.dma_start(ones[:], ones_d[:])
        onescol = ones[:, 0:1]
        onesrow = ones[0:1, :]
        onesb = pp.tile([128, 520], BF)        # bf16 ones
        nc.sync.dma_start(onesb[:], onesb_d[:])
        obcol = onesb[:, 0:1]
        obrow = onesb[0:1, :]
        mask = [pp.tile([128, H], BF, tag=f"mask{t}", name=f"mask{t}") for t in range(KT)]
        for t in range(KT):
            nc.sync.dma_start(mask[t][:], maskneg_d[128 * t:128 * (t + 1), :])
        clspb = [pp.tile([128, 4], F32R, tag=f"clspb{t}", name=f"clspb{t}") for t in range(KT)]
        for t in range(KT):
            nc.sync.dma_start(clspb[t][:], clspb_d[128 * t:128 * (t + 1), :])
        ident = pp.tile([128, 128], F32)
        make_identity(nc, ident[:])
        # resident rel-pos-bias (bf16), loaded once and reused every layer
        rpb_s = [(pp.tile([128, NP256], BF, tag=f"rpba{h}", name=f"rpba{h}"),
                  pp.tile([N - 128, NP256], BF, tag=f"rpbb{h}", name=f"rpbb{h}"))
                 for h in range(H)]
        for h in range(H):
            nc.sync.dma_start(rpb_s[h][0][:], rpbT_d[h, 0:128, :])
            nc.sync.dma_start(rpb_s[h][1][:], rpbT_d[h, 128:N, :])

        # ---- persistent state / per-layer reused buffers ----
        xs = [pp.tile([128, T2], F32R, tag=f"xs{t}", name=f"xs{t}") for t in range(KT)]
        xn = [pp.tile([128, T2], BF, tag=f"xn{t}", name=f"xn{t}") for t in range(KT)]
        xsq = [pp.tile([128, T2], F32R, tag=f"xsq{t}", name=f"xsq{t}") for t in range(KT)]
        qsq = [pp.tile([128, T2], BF, tag=f"qsq{t}", name=f"qsq{t}") for t in range(KT)]
        qkvs = [pp.tile([128, T2], BF, tag=f"qkvs{t}", name=f"qkvs{t}") for t in range(2 * KT)]
        sqk = [pp.tile([128, NP256], BF, tag=f"sqk{t}", name=f"sqk{t}") for t in range(2 * KT)]
        vtm = {}
        for path in (0, 1):
            vtm[path] = (pp.tile([128, E], BF, tag=f"vtm{path}a", name=f"vtm{path}a"),
                         pp.tile([N - 128, E], BF, tag=f"vtm{path}b", name=f"vtm{path}b"))
        ctx_s = [pp.tile([128, T2], BF, tag=f"ctx{t}", name=f"ctx{t}") for t in range(KT)]
        gath = pp.tile([1, H * N + 64], BF)
        nc.sync.dma_start(gath[:], zeros_d[:])   # zero the pad columns once
        an_t = pp.tile([H, N], BF)
        bn_t = pp.tile([H, N], F32)
        bT = (pp.tile([128, H], F32, tag="bT0", name="bT0"), pp.tile([N - 128, H], F32, tag="bT1", name="bT1"))
        stage = pp.tile([64, T2], BF)
        eps_t = pp.tile([1, 1], F32)
        nc.vector.memset(eps_t[:], 1e-5)

        MTOK = (128, N - 128)      # token m-tile sizes
        eps = 1e-5

        def layer_norm(src, g_ap, b_ap):
            """src: 6 [128,T2] f32r tiles -> xn (feature-major LN over partitions)."""
            for t in range(KT):
                nc.scalar.activation(xsq[t][:], src[t][:], AF.Square)
            p_s = bank([1, T2])
            p_s2 = bank([1, T2])
            for t in range(KT):
                nc.tensor.matmul(p_s[:], onescol, src[t][:],
                                 start=(t == 0), stop=(t == KT - 1))
            for t in range(KT):
                nc.tensor.matmul(p_s2[:], onescol, xsq[t][:],
                                 start=(t == 0), stop=(t == KT - 1))
            mu = rp.tile([1, T2], F32, tag="ln_mu")
            nc.vector.tensor_scalar(mu[:], p_s[:], 1.0 / E, None, ALU.mult)
            var = rp.tile([1, T2], F32, tag="ln_var")
            nc.vector.tensor_scalar(var[:], p_s2[:], 1.0 / E, None, ALU.mult)
            musq = rp.tile([1, T2], F32, tag="ln_musq")
            nc.scalar.activation(musq[:], mu[:], AF.Square)
            nc.vector.tensor_sub(var[:], var[:], musq[:])
            std = rp.tile([1, T2], F32, tag="ln_std")
            nc.scalar.activation(std[:], var[:], AF.Sqrt, bias=eps_t[:])
            rinv = rp.tile([1, T2], F32, tag="ln_rinv")
            nc.vector.reciprocal(rinv[:], std[:])
            rinv_r = rp.tile([1, T2], F32R, tag="ln_rinvr")
            nc.scalar.activation(rinv_r[:], rinv[:], AF.Copy)
            mur = rp.tile([1, T2], F32R, tag="ln_mur")
            nc.vector.tensor_mul(mur[:], mu[:], rinv_r[:])
            p_R = bank([128, T2])
            p_MR = bank([128, T2])
            nc.tensor.matmul(p_R[:], onesrow[:, 0:128], rinv_r[:], start=True, stop=True)
            nc.tensor.matmul(p_MR[:], onesrow[:, 0:128], mur[:], start=True, stop=True)
            for t in range(KT):
                tmp = bp.tile([128, T2], F32, tag="ln_tmp")
                nc.vector.tensor_mul(tmp[:], src[t][:], p_R[:])
                nc.vector.tensor_sub(tmp[:], tmp[:], p_MR[:])
                nc.scalar.activation(xn[t][:], tmp[:], AF.Identity,
                                     bias=b_ap(t), scale=g_ap(t))

        for _rep in range(repeats):
            # ================= patch embed =================
            xcol = [pp.tile([128, NP256], BF, tag=f"xcol{t}", name=f"xcol{t}") for t in range(KT)]
            for t in range(KT):
                nc.sync.dma_start(xcol[t][:], xcol_d[128 * t:128 * (t + 1), :])

            for path, w_d in ((0, pw_d), (1, cpw_d)):
                wt = []
                for kt in range(KT):
                    w = wchunk()
                    nc.sync.dma_start(w[:], w_d[128 * kt:128 * (kt + 1), :])
                    wt.append(w)
                for mt in range(KT):
                    pe = bank([128, NP256])
                    for kt in range(KT):
                        nc.tensor.matmul(pe[:], wt[kt][:, 128 * mt:128 * (mt + 1)],
                                         xcol[kt][:], start=(kt == 0), stop=(kt == KT - 1))
                    dst = xs[mt][:, 1 + path * N: 1 + path * N + NPATCH]
                    nc.scalar.activation(dst, pe[:, 0:NPATCH], AF.Identity,
                                         bias=clspb[mt][:, 2 + path: 3 + path], scale=1.0)
            # cls tokens into cols 0 / 197
            for t in range(KT):
                nc.vector.tensor_copy(
                    xs[t][:, 0:T2].rearrange("p (a c) -> p a c", a=2)[:, :, 0:1],
                    clspb[t][:, 0:2].rearrange("p (a c) -> p a c", a=2)[:, :, 0:1])

            if debug:
                for t in range(KT):
                    nc.sync.dma_start(dbg_d[0, 128 * t:128 * (t + 1), :], xs[t][:])

            # ================= transformer layers =================
            for li in range(L):
                A = rp.tile([128, 36], F32, tag="acols")
                nc.sync.dma_start(A[:], acols_d[li])
                Bq = rp.tile([128, 24], F32, tag="bqcols")
                nc.sync.dma_start(Bq[:], bq_d[li])
                f1b = rp.tile([128, 24], F32, tag="fc1b")
                nc.sync.dma_start(f1b[:], fc1b_d[li])
                vb = rp.tile([1, 2 * E], BF, tag="vbrow")
                nc.sync.dma_start(vb[:], vb_d[li])
                pbr = rp.tile([1, 3 * E], BF, tag="pbrow")
                nc.sync.dma_start(pbr[:], pbrow_d[li])

                def ac(t, c):
                    return A[:, 6 * t + c: 6 * t + c + 1]

                # ---- LN1 ----
                layer_norm(xs, lambda t: ac(t, 0), lambda t: ac(t, 1))

                # ---- qkv (q|k part), feature-major, both paths at once ----
                qkw = {}
                for b in (0, 1):
                    for kt in range(KT):
                        w = wchunk()
                        nc.sync.dma_start(w[:], qkvw_d[li, 128 * kt:128 * (kt + 1),
                                                       E * b:E * (b + 1)])
                        qkw[(b, kt)] = w
                for mt in range(2 * KT):
                    b, m = mt // KT, mt % KT
                    pq = bank([128, T2])
                    for kt in range(KT):
                        nc.tensor.matmul(pq[:], qkw[(b, kt)][:, 128 * m:128 * (m + 1)],
                                         xn[kt][:], start=(kt == 0), stop=(kt == KT - 1))
                    # mean columns
                    if mt < KT:   # q rows: * 2*SCALE, + 2*SCALE*q_bias
                        nc.vector.tensor_scalar(qkvs[mt][:, 0:N], pq[:, 0:N],
                                                2.0 * SCALE, Bq[:, 2 * mt:2 * mt + 1],
                                                ALU.mult, ALU.add)
                    else:
                        nc.vector.tensor_copy(qkvs[mt][:, 0:N], pq[:, 0:N])
                    # cov columns: elu(x + b) + 1 = exp(min(x+b,0)) + max(x+b,0)
                    e1 = bp.tile([128, N], F32, tag="elu1")
                    e2 = bp.tile([128, N], F32, tag="elu2")
                    bcol = Bq[:, 2 * mt + 1:2 * mt + 2]
                    nc.vector.tensor_scalar(e1[:], pq[:, N:T2], bcol, 0.0, ALU.add, ALU.min)
                    nc.scalar.activation(e1[:], e1[:], AF.Exp)
                    nc.vector.tensor_scalar(e2[:], pq[:, N:T2], bcol, 0.0, ALU.add, ALU.max)
                    nc.vector.tensor_add(qkvs[mt][:, N:T2], e1[:], e2[:])

                # sq = 2*sqrt(cq) = sqrt(4 cq); sk = sqrt(ck)
                for mt in range(2 * KT):
                    nc.scalar.activation(sqk[mt][:, 0:N], qkvs[mt][:, N:T2], AF.Sqrt,
                                         scale=4.0 if mt < KT else 1.0)

                # ---- v token-major (both paths), weight as moving operand ----
                for kt in range(KT):
                    w = wchunk()
                    nc.sync.dma_start(w[:], qkvw_d[li, 128 * kt:128 * (kt + 1), 2 * E:3 * E])
                    qkw[(2, kt)] = w
                for path in (0, 1):
                    for Mt in range(2):
                        moff, mw = 128 * Mt, MTOK[Mt]
                        for ch in range(2):
                            pv = bank([mw, 384])
                            for kt in range(KT):
                                nc.tensor.matmul(
                                    pv[:], xn[kt][:, path * N + moff: path * N + moff + mw],
                                    qkw[(2, kt)][:, 384 * ch:384 * (ch + 1)],
                                    start=(kt == 0), stop=False)
                            nc.tensor.matmul(pv[:], obrow[:, 0:mw],
                                             vb[:, path * E + 384 * ch: path * E + 384 * (ch + 1)],
                                             start=False, stop=True)
                            dst = vtm[path][Mt][:, 384 * ch:384 * (ch + 1)]
                            if path == 0:
                                nc.vector.tensor_copy(dst, pv[:])
                            else:
                                e1 = bp.tile([128, 384], F32, tag="velu1")
                                e2 = bp.tile([128, 384], F32, tag="velu2")
                                nc.vector.tensor_scalar(e1[0:mw, :], pv[:], 0.0, None, ALU.min)
                                nc.scalar.activation(e1[0:mw, :], e1[0:mw, :], AF.Exp)
                                nc.vector.tensor_scalar(e2[0:mw, :], pv[:], 0.0, None, ALU.max)
                                nc.vector.tensor_add(dst, e1[0:mw, :], e2[0:mw, :])

                # ---- a/b norm terms via -1-blockdiag mask matmuls ----
                p_a = bank([H, T2])
                p_b = bank([H, T2])
                for half, pdst in ((0, p_a), (1, p_b)):
                    for kt in range(KT):
                        src = qkvs[half * KT + kt]
                        nc.scalar.activation(qsq[kt][:, 0:N], src[:, 0:N], AF.Square,
                                             scale=0.5 if half == 0 else 1.0)
                        nc.vector.tensor_copy(qsq[kt][:, N:T2], src[:, N:T2])
                    for kt in range(KT):
                        nc.tensor.matmul(pdst[:], mask[kt][:], qsq[kt][:],
                                         start=(kt == 0), stop=(kt == KT - 1))
                nc.vector.tensor_copy(an_t[:], p_a[:, 0:N])
                nc.vector.tensor_add(an_t[:], an_t[:], p_a[:, N:T2])
                nc.vector.tensor_copy(bn_t[:], p_b[:, 0:N])
                nc.vector.tensor_add(bn_t[:], bn_t[:], p_b[:, N:T2])
                # gather -a rows to partition 0 (strided per-head 256 slots)
                nc.sync.dma_start(
                    gath[:, 0:H * N].rearrange("p (h c) -> p h c", c=N), an_t[:])
                # transpose -b to per-token columns
                for Mt in range(2):
                    moff, mw = 128 * Mt, MTOK[Mt]
                    pt = bank([mw, H])
                    nc.tensor.transpose(pt[:], bn_t[:, moff:moff + mw], ident[0:H, 0:H])
                    nc.vector.tensor_copy(bT[Mt][:], pt[:])

                # ---- attention, head by head ----
                for h in range(H):
                    qt, off = h // 2, 64 * (h % 2)
                    Et, E2t, psc = [], [], []
                    for Mt in range(2):
                        moff, mw = 128 * Mt, MTOK[Mt]
                        pc = bank([mw, NP256])
                        nc.tensor.matmul(pc[:], qkvs[KT + qt][off:off + 64, moff:moff + mw],
                                         qkvs[qt][off:off + 64, 0:NP256],
                                         start=True, stop=False)
                        nc.tensor.matmul(pc[:], sqk[KT + qt][off:off + 64, moff:moff + mw],
                                         sqk[qt][off:off + 64, 0:NP256],
                                         start=False, stop=False)
                        nc.tensor.matmul(pc[:], obrow[:, 0:mw],
                                         gath[:, N * h:N * h + NP256],
                                         start=False, stop=True)
                        psc.append(pc)
                    for Mt in range(2):
                        mw = MTOK[Mt]
                        sg = ap.tile([128, NP256], BF, tag="sig")
                        nc.scalar.activation(sg[0:mw, :], psc[Mt][:], AF.Sigmoid,
                                             bias=bT[Mt][:, h:h + 1])
                        nc.vector.tensor_add(sg[0:mw, :], sg[0:mw, :], rpb_s[h][Mt][:])
                        Ee = ap.tile([128, NP256], BF, tag="E")
                        nc.scalar.activation(Ee[0:mw, :], sg[0:mw, :], AF.Exp)
                        Et.append(Ee)
                    pd = bank([1, NP256])
                    for Mt in range(2):
                        mw = MTOK[Mt]
                        nc.tensor.matmul(pd[:], obcol[0:mw, :], Et[Mt][0:mw, :],
                                         start=(Mt == 0), stop=(Mt == 1))
                    rd = ap.tile([1, NP256], F32, tag="rd", bufs=2)
                    nc.vector.reciprocal(rd[:], pd[:])
                    rdr = ap.tile([1, NP256], BF, tag="rdr", bufs=2)
                    nc.scalar.activation(rdr[:], rd[:], AF.Copy)
                    pr = bank([128, NP256])
                    nc.tensor.matmul(pr[:], obrow[:, 0:128], rdr[:], start=True, stop=True)
                    for Mt in range(2):
                        mw = MTOK[Mt]
                        nc.vector.tensor_mul(Et[Mt][0:mw, :], Et[Mt][0:mw, :], pr[0:mw, :])
                        E2 = ap.tile([128, NP256], BF, tag="E2")
                        nc.vector.tensor_mul(E2[0:mw, :], Et[Mt][0:mw, :], Et[Mt][0:mw, :])
                        E2t.append(E2)
                    pctx = bank([64, 512])
                    for path in (0, 1):
                        src = Et if path == 0 else E2t
                        for Mt in range(2):
                            mw = MTOK[Mt]
                            nc.tensor.matmul(pctx[:, 256 * path:256 * (path + 1)],
                                             vtm[path][Mt][:, 64 * h:64 * (h + 1)],
                                             src[Mt][0:mw, :],
                                             start=(Mt == 0), stop=(Mt == 1))
                    pv = pctx[:, 0:512].rearrange("p (a c) -> p a c", a=2)[:, :, 0:N]
                    if off == 0:
                        nc.vector.tensor_copy(
                            ctx_s[qt][0:64, 0:T2].rearrange("p (a c) -> p a c", a=2), pv)
                    else:
                        nc.vector.tensor_copy(
                            stage[:, 0:T2].rearrange("p (a c) -> p a c", a=2), pv)
                        nc.sync.dma_start(ctx_s[qt][64:128, :], stage[:])

                # ---- proj / cov_proj + gamma1-scaled residual ----
                for path, w_d in ((0, projw_d), (1, cprojw_d)):
                    pwt = []
                    for kt in range(KT):
                        w = wchunk()
                        nc.sync.dma_start(w[:], w_d[li, 128 * kt:128 * (kt + 1), :])
                        pwt.append(w)
                    win = 0 if path == 0 else COVW
                    vo = 0 if path == 0 else COVO
                    for mt in range(KT):
                        pj = bank([128, NP256])
                        for kt in range(KT):
                            nc.tensor.matmul(pj[:], pwt[kt][:, 128 * mt:128 * (mt + 1)],
                                             ctx_s[kt][:, win:win + NP256],
                                             start=(kt == 0), stop=False)
                        nc.tensor.matmul(pj[:], pbr[:, path * E + 128 * mt:
                                                    path * E + 128 * (mt + 1)],
                                         obrow[:, 0:NP256], start=False, stop=True)
                        nc.vector.scalar_tensor_tensor(
                            xs[mt][:, path * N:(path + 1) * N], pj[:, vo:vo + N],
                            ac(mt, 4), xs[mt][:, path * N:(path + 1) * N],
                            ALU.mult, ALU.add)

                # ---- LN2 + MLP (chunked fc2 accumulation) ----
                layer_norm(xs, lambda t: ac(t, 2), lambda t: ac(t, 3))
                pf2 = [ps.tile([128, T2], F32, tag="bank", name=f"pf2_{_i}") for _i in range(KT)]
                f1w = {}

                def load_f1(jb):
                    for kt in range(KT):
                        w = wchunk()
                        nc.sync.dma_start(w[:], fc1w_d[li, 128 * kt:128 * (kt + 1),
                                                       E * jb:E * (jb + 1)])
                        f1w[(jb, kt)] = w

                load_f1(0)
                for j in range(MT_H):
                    jb, jm = j // KT, j % KT
                    if jm == 0 and jb + 1 < 4:
                        load_f1(jb + 1)
                    ph = bank([128, T2])
                    for kt in range(KT):
                        nc.tensor.matmul(ph[:], f1w[(jb, kt)][:, 128 * jm:128 * (jm + 1)],
                                         xn[kt][:], start=(kt == 0), stop=(kt == KT - 1))
                    Hj = bp.tile([128, T2], BF, tag="hj")
                    nc.scalar.activation(Hj[:], ph[:], AF.Gelu, bias=f1b[:, j:j + 1])
                    w2 = wchunk()
                    nc.sync.dma_start(w2[:], fc2w_d[li, 128 * j:128 * (j + 1), :])
                    for i in range(KT):
                        nc.tensor.matmul(pf2[i][:], w2[:, 128 * i:128 * (i + 1)], Hj[:],
                                         start=(j == 0), stop=False,
                                         skip_group_check=True)
                for i in range(KT):
                    nc.tensor.matmul(pf2[i][:], pbr[:, 2 * E + 128 * i:2 * E + 128 * (i + 1)],
                                     obrow[:, 0:T2], start=False, stop=True,
                                     skip_group_check=True)
                    nc.vector.scalar_tensor_tensor(xs[i][:], pf2[i][:], ac(i, 5), xs[i][:],
                                                   ALU.mult, ALU.add)

                if debug:
                    for t in range(KT):
                        nc.sync.dma_start(dbg_d[li + 1, 128 * t:128 * (t + 1), :], xs[t][:])

            # ================= head =================
            pl = [rp.tile([128, 2], F32R, tag=f"pool{t}", name=f"pool{t}") for t in range(KT)]
            for t in range(KT):
                nc.vector.tensor_reduce(pl[t][:, 0:1], xs[t][:, 1:N], mybir.AxisListType.X,
                                        ALU.add)
                nc.vector.tensor_scalar(pl[t][:, 0:1], pl[t][:, 0:1], 1.0 / NPATCH,
                                        None, ALU.mult)
                nc.scalar.activation(pl[t][:, 1:2], pl[t][:, 0:1], AF.Square)
            p_s = bank([1, 2])
            for t in range(KT):
                nc.tensor.matmul(p_s[:], onescol, pl[t][:],
                                 start=(t == 0), stop=(t == KT - 1))
            mu = rp.tile([1, 2], F32, tag="hmu")
            nc.vector.tensor_scalar(mu[:], p_s[:], 1.0 / E, None, ALU.mult)
            musq = rp.tile([1, 1], F32, tag="hmusq")
            nc.scalar.activation(musq[:], mu[:, 0:1], AF.Square)
            var = rp.tile([1, 1], F32, tag="hvar")
            nc.vector.tensor_sub(var[:], mu[:, 1:2], musq[:])
            std = rp.tile([1, 1], F32, tag="hstd")
            nc.scalar.activation(std[:], var[:], AF.Sqrt, bias=eps_t[:])
            rinv = rp.tile([1, 1], F32, tag="hrinv")
            nc.vector.reciprocal(rinv[:], std[:])
            rr = rp.tile([1, 2], F32R, tag="hrr")
            nc.scalar.activation(rr[:, 0:1], rinv[:], AF.Copy)
            nc.vector.tensor_mul(rr[:, 1:2], mu[:, 0:1], rr[:, 0:1])
            p_bc = bank([128, 2])
            nc.tensor.matmul(p_bc[:], onesrow[:, 0:128], rr[:], start=True, stop=True)
            fcn = [rp.tile([128, 2], F32, tag=f"fcn{t}", name=f"fcn{t}") for t in range(KT)]
            tn = [rp.tile([128, 2], BF, tag=f"tn{t}", name=f"tn{t}") for t in range(KT)]
            for t in range(KT):
                nc.sync.dma_start(fcn[t][:], fcn_d[128 * t:128 * (t + 1), :])
                tmp = rp.tile([128, 1], F32, tag="htmp")
                nc.vector.tensor_mul(tmp[:], pl[t][:, 0:1], p_bc[:, 0:1])
                nc.vector.tensor_sub(tmp[:], tmp[:], p_bc[:, 1:2])
                nc.scalar.activation(tn[t][:, 0:1], tmp[:], AF.Identity,
                                     bias=fcn[t][:, 1:2], scale=fcn[t][:, 0:1])
                nc.vector.tensor_copy(tn[t][:, 1:2], tn[t][:, 0:1])
            hwt = {}
            for blk in range(2):
                for kt in range(KT):
                    w = wchunk()
                    wd = 768 if blk == 0 else 256
                    nc.sync.dma_start(w[:, 0:wd],
                                      headw_d[128 * kt:128 * (kt + 1),
                                              768 * blk:768 * blk + wd])
                    hwt[(blk, kt)] = w
            hb = rp.tile([128, 8], F32, tag="headb")
            nc.sync.dma_start(hb[:], headb_d[:])
            osb = rp.tile([128, 8], F32, tag="osb")
            for mt in range(8):
                blk, mo = (0, mt) if mt < 6 else (1, mt - 6)
                po = bank([128, 2])
                for kt in range(KT):
                    nc.tensor.matmul(po[:], hwt[(blk, kt)][:, 128 * mo:128 * (mo + 1)],
                                     tn[kt][:], start=(kt == 0), stop=(kt == KT - 1))
                nc.scalar.activation(osb[:, mt:mt + 1], po[:, 0:1], AF.Identity,
                                     bias=hb[:, mt:mt + 1])
            nc.sync.dma_start(out_d[:].rearrange("(a p) c -> p a c", p=128),
                              osb[:].rearrange("p (a c) -> p a c", c=1))

    lp.__exit__(None, None, None)
    nc.finalize()
    return nc


# --------------------------------------------------------------------------
# host-side input preparation
# --------------------------------------------------------------------------

def prep_shared(i):
    """Build the shared (weights etc.) input map from the full input dict."""
    import ml_dtypes
    f = np.float32
    bf = ml_dtypes.bfloat16

    def g(k):
        return np.asarray(i[k], dtype=f)

    qkvw = np.ascontiguousarray(np.transpose(g("qkv_w"), (0, 2, 1))).astype(bf)
    projw = np.ascontiguousarray(np.transpose(g("proj_w"), (0, 2, 1))).astype(bf)
    cprojw = np.ascontiguousarray(np.transpose(g("cov_proj_w"), (0, 2, 1))).astype(bf)
    fc1w = np.ascontiguousarray(np.transpose(g("fc1_w"), (0, 2, 1))).astype(bf)
    fc2w = np.ascontiguousarray(np.transpose(g("fc2_w"), (0, 2, 1))).astype(bf)
    pw = np.ascontiguousarray(g("patch_w").reshape(E, E).T).astype(bf)
    cpw = np.ascontiguousarray(g("cov_patch_w").reshape(E, E).T).astype(bf)
    headw = np.zeros((E, 1024), bf)
    headw[:, 0:NCLS] = g("head_w").T.astype(bf)
    rpbT = np.zeros((H, N, NP256), bf)
    rpbT[:, :, 0:N] = np.transpose(g("rel_pos_bias"), (0, 2, 1)).astype(bf)

    acols = np.zeros((L, 128, 36), f)
    for c, k in enumerate(["norm1_g", "norm1_b", "norm2_g", "norm2_b",
                           "gamma1", "gamma2"]):
        v = g(k).reshape(L, KT, 128)
        for t in range(KT):
            acols[:, :, 6 * t + c] = v[:, t, :]
    bq = np.zeros((L, 128, 24), f)
    qb2 = (2.0 * SCALE) * g("q_bias")
    cqb = g("cov_q_bias")
    for mt in range(KT):
        bq[:, :, 2 * mt] = qb2[:, 128 * mt:128 * (mt + 1)]
        bq[:, :, 2 * mt + 1] = cqb[:, 128 * mt:128 * (mt + 1)]
    fc1b = np.ascontiguousarray(g("fc1_b").reshape(L, 24, 128).transpose(0, 2, 1))
    vb = np.concatenate([g("v_bias"), g("cov_v_bias")], axis=1)[:, None, :].astype(bf)
    pbrow = np.concatenate([g("proj_b"), g("cov_proj_b"), g("fc2_b")],
                           axis=1)[:, None, :].astype(bf)
    maskneg = np.zeros((E, H), bf)
    for h in range(H):
        maskneg[64 * h:64 * (h + 1), h] = -1.0
    clspb = np.zeros((E, 4), f)
    clspb[:, 0] = g("cls_tok").reshape(E)
    clspb[:, 1] = g("cov_cls_tok").reshape(E)
    clspb[:, 2] = g("patch_b")
    clspb[:, 3] = g("cov_patch_b")
    fcn = np.stack([g("fc_norm_g"), g("fc_norm_b")], axis=1)
    hbp = np.zeros(1024, f)
    hbp[0:NCLS] = g("head_b")
    headb = np.ascontiguousarray(hbp.reshape(8, 128).T)
    return {
        "qkvw": qkvw, "projw": projw, "cprojw": cprojw, "fc1w": fc1w,
        "fc2w": fc2w, "pw": pw, "cpw": cpw, "headw": headw, "rpbT": rpbT,
        "acols": acols, "bq": bq, "fc1b": fc1b, "vb": vb, "pbrow": pbrow,
        "ones": np.ones((128, 520), f), "onesb": np.ones((128, 520), bf),
        "zeros": np.zeros((1, H * N + 64), bf),
        "maskneg": maskneg, "clspb": clspb, "fcn": fcn,
        "headb": headb,
    }


def im2col(x):
    """x: [B,3,224,224] -> [B, 768, 256] (zero-padded cols, bf16)."""
    import ml_dtypes
    xc = np.asarray(x, dtype=np.float32).reshape(B, 3, 14, 16, 14, 16)
    xc = xc.transpose(0, 1, 3, 5, 2, 4).reshape(B, E, NPATCH)
    out = np.zeros((B, E, NP256), ml_dtypes.bfloat16)
    out[:, :, 0:NPATCH] = xc.astype(ml_dtypes.bfloat16)
    return out


def _get_nc(debug=False, repeats=1):
    key = ("nc", debug, repeats)
    if key not in _CACHE:
        _CACHE[key] = build_nc(debug=debug, repeats=repeats)
    return _CACHE[key]


def run(inputs, debug=False, trace=False, repeats=1, tmpdir=None):
    nc = _get_nc(debug=debug, repeats=repeats)
    shared = prep_shared(inputs)
    xcols = im2col(inputs["x"])
    in_maps = [dict(shared, xcol=np.ascontiguousarray(xcols[b])) for b in range(B)]
    res = run_bass_kernel_spmd(nc, in_maps, list(range(B)), trace=trace,
                               tmpdir=tmpdir)
    y = np.stack([res.results[b]["out"][0:NCLS, 0] for b in range(B)], axis=0)
    return y.astype(np.float32), res


def kernel(**inputs) -> np.ndarray:
    y, _ = run(inputs)
    return y



# revision 38
# speedup vs baseline: 1.1314x; 1.0252x over previous
"""DistVisionTransformer (STOSA-style ViT, mean+cov paths) on 8 Trainium2
NeuronCores. Data-parallel: one image per core; full forward pass on-device
in float32r (TF32-like) matmuls with fp32 accumulation.

Layout: activations are feature-major [768, 394] where columns 0:197 are the
mean-path tokens and 197:394 the cov-path tokens (cls token at cols 0 / 197).
LayerNorm / softmax partition-axis reductions are done with ones-vector
matmuls on the PE; per-token broadcasts with rank-1 ones outer products.
"""
import numpy as np
from contextlib import ExitStack

import concourse.bass as bass
import concourse.bacc as bacc
import concourse.tile as tile
import concourse.mybir as mybir
from concourse.bass_utils import run_bass_kernel_spmd
from concourse.masks import make_identity

F32 = mybir.dt.float32
F32R = mybir.dt.float32r
BF = mybir.dt.bfloat16
AF = mybir.ActivationFunctionType
ALU = mybir.AluOpType

B, E, H, L, P, IMG, NCLS = 8, 768, 12, 12, 16, 224, 1000
D = E // H                  # 64
MLP = 4 * E                 # 3072
SCALE = D ** -0.5
NPATCH = (IMG // P) ** 2    # 196
N = NPATCH + 1              # 197
T2 = 2 * N                  # 394  (mean | cov concatenated along tokens)
NP256 = 256                 # padded token free-dim for 256-wide matmuls
KT = E // 128               # 6 k-tiles over features
MT_H = MLP // 128           # 24 hidden tiles
COVW = T2 - NP256           # 138: start of the 256-wide cov window
COVO = N - COVW             # 59: offset of cov data inside that window

_CACHE = {}


# --------------------------------------------------------------------------
# device kernel builder
# --------------------------------------------------------------------------

def build_nc(debug=False, repeats=1):
    nc = bacc.Bacc(None, target_bir_lowering=False)
    lp = nc.allow_low_precision("tf32-style kernel; fp32 psum accumulation")
    lp.__enter__()

    dp = nc.declare_dram_parameter
    xcol_d = dp("xcol", [E, NP256], BF, isOutput=False)          # per-core im2col
    qkvw_d = dp("qkvw", [L, E, 3 * E], BF, isOutput=False)       # [in, out] (q|k|v)
    projw_d = dp("projw", [L, E, E], BF, isOutput=False)
    cprojw_d = dp("cprojw", [L, E, E], BF, isOutput=False)
    fc1w_d = dp("fc1w", [L, E, MLP], BF, isOutput=False)
    fc2w_d = dp("fc2w", [L, MLP, E], BF, isOutput=False)
    pw_d = dp("pw", [E, E], BF, isOutput=False)                  # patch embed [in, out]
    cpw_d = dp("cpw", [E, E], BF, isOutput=False)
    headw_d = dp("headw", [E, 1024], BF, isOutput=False)         # [in, out] padded
    rpbT_d = dp("rpbT", [H, N, NP256], BF, isOutput=False)       # rpb[h].T, padded
    acols_d = dp("acols", [L, 128, 36], F32, isOutput=False)     # per-tile param cols
    bq_d = dp("bq", [L, 128, 24], F32, isOutput=False)           # qkv psum bias cols
    fc1b_d = dp("fc1b", [L, 128, 24], F32, isOutput=False)
    vb_d = dp("vb", [L, 1, 2 * E], BF, isOutput=False)           # v / cov_v bias rows
    pbrow_d = dp("pbrow", [L, 1, 3 * E], BF, isOutput=False)     # proj|cproj|fc2 bias rows
    ones_d = dp("ones", [128, 520], F32R, isOutput=False)        # all-ones block
    onesb_d = dp("onesb", [128, 520], BF, isOutput=False)        # all-ones block bf16
    zeros_d = dp("zeros", [1, H * N + 64], BF, isOutput=False)
    maskneg_d = dp("maskneg", [E, H], BF, isOutput=False)        # -1 blockdiag
    clspb_d = dp("clspb", [E, 4], F32R, isOutput=False)          # cls|cov_cls|patch_b|cov_patch_b
    headb_d = dp("headb", [128, 8], F32, isOutput=False)
    out_d = dp("out", [1024, 1], F32, isOutput=True)
    if debug:
        dbg_d = dp("dbg", [L + 1, E, T2], F32R, isOutput=True)

    with tile.TileContext(nc) as tc, ExitStack() as ctx:
        pp = ctx.enter_context(tc.tile_pool(name="persist", bufs=1))
        wp = ctx.enter_context(tc.tile_pool(name="weights", bufs=14))
        rp = ctx.enter_context(tc.tile_pool(name="rows", bufs=1))
        bp = ctx.enter_context(tc.tile_pool(name="bigscratch", bufs=2))
        ap = ctx.enter_context(tc.tile_pool(name="attn", bufs=2))
        ps = ctx.enter_context(tc.tile_pool(name="psum", bufs=8, space="PSUM"))

        _bn = [0]

        def bank(shape):
            _bn[0] += 1
            return ps.tile(shape, F32, tag="bank", name=f"pb{_bn[0]}")

        _wn = [0]

        def wchunk():
            _wn[0] += 1
            return wp.tile([128, E], BF, tag="wchunk", name=f"w{_wn[0]}")

        # ---- persistent constants ----
        ones = pp.tile([128, 520], F32R)       # columns / rows of ones (f32r)
        nc.sync.dma_start(ones[:], ones_d[:])
        onescol = ones[:, 0:1]
        onesrow = ones[0:1, :]
        onesb = pp.tile([128, 520], BF)        # bf16 ones
        nc.sync.dma_start(onesb[:], onesb_d[:])
        obcol = onesb[:, 0:1]
        obrow = onesb[0:1, :]
        mask = [pp.tile([128, H], BF, tag=f"mask{t}", name=f"mask{t}") for t in range(KT)]
        for t in range(KT):
            nc.sync.dma_start(mask[t][:], maskneg_d[128 * t:128 * (t + 1), :])
        clspb = [pp.tile([128, 4], F32R, tag=f"clspb{t}", name=f"clspb{t}") for t in range(KT)]
        for t in range(KT):
            nc.sync.dma_start(clspb[t][:], clspb_d[128 * t:128 * (t + 1), :])
        ident = pp.tile([128, 128], F32)
        make_identity(nc, ident[:])
        # resident rel-pos-bias (bf16), loaded once and reused every layer
        rpb_s = [(pp.tile([128, NP256], BF, tag=f"rpba{h}", name=f"rpba{h}"),
                  pp.tile([N - 128, NP256], BF, tag=f"rpbb{h}", name=f"rpbb{h}"))
                 for h in range(H)]
        for h in range(H):
            nc.sync.dma_start(rpb_s[h][0][:], rpbT_d[h, 0:128, :])
            nc.sync.dma_start(rpb_s[h][1][:], rpbT_d[h, 128:N, :])

        # ---- persistent state / per-layer reused buffers ----
        xs = [pp.tile([128, T2], F32R, tag=f"xs{t}", name=f"xs{t}") for t in range(KT)]
        xn = [pp.tile([128, T2], BF, tag=f"xn{t}", name=f"xn{t}") for t in range(KT)]
        xsq = [pp.tile([128, T2], F32R, tag=f"xsq{t}", name=f"xsq{t}") for t in range(KT)]
        qsq = [pp.tile([128, T2], BF, tag=f"qsq{t}", name=f"qsq{t}") for t in range(2 * KT)]
        qkvs = [pp.tile([128, T2], BF, tag=f"qkvs{t}", name=f"qkvs{t}") for t in range(2 * KT)]
        sqk = [pp.tile([128, NP256], BF, tag=f"sqk{t}", name=f"sqk{t}") for t in range(2 * KT)]
        vtm = {}
        for path in (0, 1):
            vtm[path] = (pp.tile([128, E], BF, tag=f"vtm{path}a", name=f"vtm{path}a"),
                         pp.tile([N - 128, E], BF, tag=f"vtm{path}b", name=f"vtm{path}b"))
        ctx_s = [pp.tile([128, T2], BF, tag=f"ctx{t}", name=f"ctx{t}") for t in range(KT)]
        gath = pp.tile([1, H * N + 64], BF)
        nc.sync.dma_start(gath[:], zeros_d[:])   # zero the pad columns once
        an_t = pp.tile([H, N], BF)
        bn_t = pp.tile([H, N], F32)
        bT = (pp.tile([128, H], F32, tag="bT0", name="bT0"), pp.tile([N - 128, H], F32, tag="bT1", name="bT1"))
        stage = pp.tile([64, T2], BF)
        eps_t = pp.tile([1, 1], F32)
        nc.vector.memset(eps_t[:], 1e-5)
        ln2_t = pp.tile([128, 1], F32)
        nc.vector.memset(ln2_t[:], float(np.log(2.0)))

        MTOK = (128, N - 128)      # token m-tile sizes
        eps = 1e-5

        def layer_norm(src):
            """src: 6 [128,T2] f32r tiles -> xn bf16 ((x-mu)*rstd; affine folded
            into the consuming weights host-side). rstd via ln/exp (natlog set)."""
            for t in range(KT):
                nc.scalar.activation(xsq[t][:], src[t][:], AF.Square)
            p_s = bank([1, T2])
            p_s2 = bank([1, T2])
            for t in range(KT):
                nc.tensor.matmul(p_s[:], onescol, src[t][:],
                                 start=(t == 0), stop=(t == KT - 1))
            for t in range(KT):
                nc.tensor.matmul(p_s2[:], onescol, xsq[t][:],
                                 start=(t == 0), stop=(t == KT - 1))
            mu = rp.tile([1, T2], F32, tag="ln_mu")
            nc.vector.tensor_scalar(mu[:], p_s[:], 1.0 / E, None, ALU.mult)
            var = rp.tile([1, T2], F32, tag="ln_var")
            nc.vector.tensor_scalar(var[:], p_s2[:], 1.0 / E, None, ALU.mult)
            musq = rp.tile([1, T2], F32, tag="ln_musq")
            nc.scalar.activation(musq[:], mu[:], AF.Square)
            nc.vector.tensor_sub(var[:], var[:], musq[:])
            lnv = rp.tile([1, T2], F32, tag="ln_lnv")
            nc.scalar.activation(lnv[:], var[:], AF.Ln, bias=eps_t[:])
            rinv_r = rp.tile([1, T2], F32R, tag="ln_rinvr")
            nc.scalar.activation(rinv_r[:], lnv[:], AF.Exp, scale=-0.5)
            mur = rp.tile([1, T2], F32R, tag="ln_mur")
            nc.vector.tensor_mul(mur[:], mu[:], rinv_r[:])
            p_R = bank([128, T2])
            p_MR = bank([128, T2])
            nc.tensor.matmul(p_R[:], onesrow[:, 0:128], rinv_r[:], start=True, stop=True)
            nc.tensor.matmul(p_MR[:], onesrow[:, 0:128], mur[:], start=True, stop=True)
            for t in range(KT):
                tmp = bp.tile([128, T2], F32, tag="ln_tmp")
                nc.vector.tensor_mul(tmp[:], src[t][:], p_R[:])
                nc.vector.tensor_sub(xn[t][:], tmp[:], p_MR[:])

        for _rep in range(repeats):
            # ================= patch embed =================
            xcol = [pp.tile([128, NP256], BF, tag=f"xcol{t}", name=f"xcol{t}") for t in range(KT)]
            for t in range(KT):
                nc.sync.dma_start(xcol[t][:], xcol_d[128 * t:128 * (t + 1), :])

            for path, w_d in ((0, pw_d), (1, cpw_d)):
                wt = []
                for kt in range(KT):
                    w = wchunk()
                    nc.sync.dma_start(w[:], w_d[128 * kt:128 * (kt + 1), :])
                    wt.append(w)
                for mt in range(KT):
                    pe = bank([128, NP256])
                    for kt in range(KT):
                        nc.tensor.matmul(pe[:], wt[kt][:, 128 * mt:128 * (mt + 1)],
                                         xcol[kt][:], start=(kt == 0), stop=(kt == KT - 1))
                    dst = xs[mt][:, 1 + path * N: 1 + path * N + NPATCH]
                    nc.scalar.activation(dst, pe[:, 0:NPATCH], AF.Identity,
                                         bias=clspb[mt][:, 2 + path: 3 + path], scale=1.0)
            # cls tokens into cols 0 / 197
            for t in range(KT):
                nc.vector.tensor_copy(
                    xs[t][:, 0:T2].rearrange("p (a c) -> p a c", a=2)[:, :, 0:1],
                    clspb[t][:, 0:2].rearrange("p (a c) -> p a c", a=2)[:, :, 0:1])

            if debug:
                for t in range(KT):
                    nc.sync.dma_start(dbg_d[0, 128 * t:128 * (t + 1), :], xs[t][:])

            # ================= transformer layers =================
            for li in range(L):
                A = rp.tile([128, 36], F32, tag="acols")
                nc.sync.dma_start(A[:], acols_d[li])
                Bq = rp.tile([128, 24], F32, tag="bqcols")
                nc.sync.dma_start(Bq[:], bq_d[li])
                f1b = rp.tile([128, 24], F32, tag="fc1b")
                nc.sync.dma_start(f1b[:], fc1b_d[li])
                vb = rp.tile([1, 2 * E], BF, tag="vbrow")
                nc.sync.dma_start(vb[:], vb_d[li])
                pbr = rp.tile([1, 3 * E], BF, tag="pbrow")
                nc.sync.dma_start(pbr[:], pbrow_d[li])

                def ac(t, c):
                    return A[:, 6 * t + c: 6 * t + c + 1]

                # ---- LN1 (affine folded into qkv weights host-side) ----
                layer_norm(xs)

                # ---- qkv (q|k part), feature-major, both paths at once ----
                # post-processing fused per m-tile so ACT/DVE overlap the
                # remaining matmuls; sqrt done as exp(0.5*ln(.)) to stay in
                # the natural_log_exp table set (no sqrt-set thrash).
                LN2C = float(np.log(2.0))
                qkw = {}
                for b in (0, 1):
                    for kt in range(KT):
                        w = wchunk()
                        nc.sync.dma_start(w[:], qkvw_d[li, 128 * kt:128 * (kt + 1),
                                                       E * b:E * (b + 1)])
                        qkw[(b, kt)] = w
                for mt in range(2 * KT):
                    b, m = mt // KT, mt % KT
                    pq = bank([128, T2])
                    for kt in range(KT):
                        nc.tensor.matmul(pq[:], qkw[(b, kt)][:, 128 * m:128 * (m + 1)],
                                         xn[kt][:], start=(kt == 0), stop=(kt == KT - 1))
                    # mean columns (bias cols already hold folded LN biases)
                    if mt < KT:   # q rows: * 2*SCALE, + bias
                        nc.vector.tensor_scalar(qkvs[mt][:, 0:N], pq[:, 0:N],
                                                2.0 * SCALE, Bq[:, 2 * mt:2 * mt + 1],
                                                ALU.mult, ALU.add)
                        # a-term source: (q_scaled)^2 (full scale)
                        nc.scalar.activation(qsq[mt][:, 0:N], qkvs[mt][:, 0:N],
                                             AF.Square, scale=0.5)
                    else:         # k rows: + bias; b-term source: 0.5*k^2
                        nc.vector.tensor_scalar(qkvs[mt][:, 0:N], pq[:, 0:N],
                                                1.0, Bq[:, 2 * mt:2 * mt + 1],
                                                ALU.mult, ALU.add)
                        nc.scalar.activation(qsq[mt][:, 0:N], qkvs[mt][:, 0:N],
                                             AF.Square, scale=float(np.sqrt(0.5)))
                    # cov columns: elu(x + b) + 1 = exp(min(x+b,0)) + max(x+b,0)
                    e1 = bp.tile([128, N], F32, tag="elu1")
                    e2 = bp.tile([128, N], F32, tag="elu2")
                    bcol = Bq[:, 2 * mt + 1:2 * mt + 2]
                    nc.vector.tensor_scalar(e1[:], pq[:, N:T2], bcol, 0.0, ALU.add, ALU.min)
                    nc.scalar.activation(e1[:], e1[:], AF.Exp)
                    nc.vector.tensor_scalar(e2[:], pq[:, N:T2], bcol, 0.0, ALU.add, ALU.max)
                    nc.vector.tensor_add(qkvs[mt][:, N:T2], e1[:], e2[:])
                    # a/b cov source (k half-weighted for the tanh bias)
                    if mt < KT:
                        nc.vector.tensor_copy(qsq[mt][:, N:T2], qkvs[mt][:, N:T2])
                    else:
                        nc.vector.tensor_scalar(qsq[mt][:, N:T2], qkvs[mt][:, N:T2],
                                                0.5, None, ALU.mult)
                    # sq = sqrt(4*cq) = exp(0.5*ln(cq) + ln2); sk = sqrt(ck)
                    lnc = bp.tile([128, N], F32, tag="lnc")
                    nc.scalar.activation(lnc[:], qkvs[mt][:, N:T2], AF.Ln)
                    if mt < KT:
                        nc.scalar.activation(sqk[mt][:, 0:N], lnc[:], AF.Exp,
                                             scale=0.5, bias=ln2_t[:])
                    else:
                        nc.scalar.activation(sqk[mt][:, 0:N], lnc[:], AF.Exp,
                                             scale=0.5)

                # ---- a/b norm terms via -1-blockdiag mask matmuls ----
                p_a = bank([H, T2])
                p_b = bank([H, T2])
                for half, pdst in ((0, p_a), (1, p_b)):
                    for kt in range(KT):
                        nc.tensor.matmul(pdst[:], mask[kt][:], qsq[half * KT + kt][:],
                                         start=(kt == 0), stop=(kt == KT - 1))
                nc.vector.tensor_copy(an_t[:], p_a[:, 0:N])
                nc.vector.tensor_add(an_t[:], an_t[:], p_a[:, N:T2])
                nc.vector.tensor_copy(bn_t[:], p_b[:, 0:N])
                nc.vector.tensor_add(bn_t[:], bn_t[:], p_b[:, N:T2])
                # gather -a rows to partition 0 (strided per-head 256 slots)
                nc.sync.dma_start(
                    gath[:, 0:H * N].rearrange("p (h c) -> p h c", c=N), an_t[:])
                # transpose -b/2 to per-token columns (tanh bias)
                for Mt in range(2):
                    moff, mw = 128 * Mt, MTOK[Mt]
                    pt = bank([mw, H])
                    nc.tensor.transpose(pt[:], bn_t[:, moff:moff + mw], ident[0:H, 0:H])
                    nc.vector.tensor_copy(bT[Mt][:], pt[:])

                # ---- v token-major (both paths), weight as moving operand ----
                for kt in range(KT):
                    w = wchunk()
                    nc.sync.dma_start(w[:], qkvw_d[li, 128 * kt:128 * (kt + 1), 2 * E:3 * E])
                    qkw[(2, kt)] = w
                for path in (0, 1):
                    for Mt in range(2):
                        moff, mw = 128 * Mt, MTOK[Mt]
                        for ch in range(2):
                            pv = bank([mw, 384])
                            for kt in range(KT):
                                nc.tensor.matmul(
                                    pv[:], xn[kt][:, path * N + moff: path * N + moff + mw],
                                    qkw[(2, kt)][:, 384 * ch:384 * (ch + 1)],
                                    start=(kt == 0), stop=False)
                            nc.tensor.matmul(pv[:], obrow[:, 0:mw],
                                             vb[:, path * E + 384 * ch: path * E + 384 * (ch + 1)],
                                             start=False, stop=True)
                            dst = vtm[path][Mt][:, 384 * ch:384 * (ch + 1)]
                            if path == 0:
                                nc.vector.tensor_copy(dst, pv[:])
                            else:
                                e1 = bp.tile([128, 384], F32, tag="velu1")
                                e2 = bp.tile([128, 384], F32, tag="velu2")
                                nc.vector.tensor_scalar(e1[0:mw, :], pv[:], 0.0, None, ALU.min)
                                nc.scalar.activation(e1[0:mw, :], e1[0:mw, :], AF.Exp)
                                nc.vector.tensor_scalar(e2[0:mw, :], pv[:], 0.0, None, ALU.max)
                                nc.vector.tensor_add(dst, e1[0:mw, :], e2[0:mw, :])

                # ---- attention, head by head ----
                # sigmoid(s) = 0.5 + 0.5*tanh(s/2): tanh+exp share one ACT
                # table set. Softmax normalization deferred to the ctx output:
                # ctx_mean *= 1/d, ctx_cov *= 1/d^2 (per query), so the
                # reciprocal is off the per-head critical path.
                for h in range(H):
                    qt, off = h // 2, 64 * (h % 2)
                    Et, E2t, psc = [], [], []
                    for Mt in range(2):
                        moff, mw = 128 * Mt, MTOK[Mt]
                        pc = bank([mw, NP256])
                        nc.tensor.matmul(pc[:], qkvs[KT + qt][off:off + 64, moff:moff + mw],
                                         qkvs[qt][off:off + 64, 0:NP256],
                                         start=True, stop=False)
                        nc.tensor.matmul(pc[:], sqk[KT + qt][off:off + 64, moff:moff + mw],
                                         sqk[qt][off:off + 64, 0:NP256],
                                         start=False, stop=False)
                        nc.tensor.matmul(pc[:], obrow[:, 0:mw],
                                         gath[:, N * h:N * h + NP256],
                                         start=False, stop=True)
                        psc.append(pc)
                    for Mt in range(2):
                        mw = MTOK[Mt]
                        sg = ap.tile([128, NP256], BF, tag="sig", bufs=3)
                        nc.scalar.activation(sg[0:mw, :], psc[Mt][:], AF.Tanh,
                                             bias=bT[Mt][:, h:h + 1], scale=0.5)
                        nc.vector.tensor_add(sg[0:mw, :], sg[0:mw, :], rpb_s[h][Mt][:])
                        Ee = ap.tile([128, NP256], BF, tag="E", bufs=3)
                        # exp(0.5*sg) = exp(sigmoid(s)+rpb)/e^0.5: the constant
                        # factor cancels in the deferred normalization.
                        nc.scalar.activation(Ee[0:mw, :], sg[0:mw, :], AF.Exp,
                                             scale=0.5)
                        Et.append(Ee)
                    pd = bank([1, NP256])
                    for Mt in range(2):
                        mw = MTOK[Mt]
                        nc.tensor.matmul(pd[:], obcol[0:mw, :], Et[Mt][0:mw, :],
                                         start=(Mt == 0), stop=(Mt == 1))
                    for Mt in range(2):
                        mw = MTOK[Mt]
                        E2 = ap.tile([128, NP256], BF, tag="E2", bufs=3)
                        nc.vector.tensor_mul(E2[0:mw, :], Et[Mt][0:mw, :], Et[Mt][0:mw, :])
                        E2t.append(E2)
                    rd = ap.tile([1, NP256], F32, tag="rd", bufs=2)
                    nc.vector.reciprocal(rd[:], pd[:])
                    rr2 = ap.tile([1, 512], BF, tag="rr2", bufs=2)
                    nc.vector.tensor_copy(rr2[:, 0:NP256], rd[:])
                    nc.vector.tensor_mul(rr2[:, NP256:512], rd[:], rd[:])
                    prh = bank([64, 512])
                    nc.tensor.matmul(prh[:], obrow[:, 0:64], rr2[:], start=True, stop=True)
                    prc = ap.tile([64, 512], BF, tag="prc", bufs=2)
                    nc.scalar.activation(prc[:], prh[:], AF.Copy)
                    pctx = bank([64, 512])
                    for path in (0, 1):
                        src = Et if path == 0 else E2t
                        for Mt in range(2):
                            mw = MTOK[Mt]
                            nc.tensor.matmul(pctx[:, 256 * path:256 * (path + 1)],
                                             vtm[path][Mt][:, 64 * h:64 * (h + 1)],
                                             src[Mt][0:mw, :],
                                             start=(Mt == 0), stop=(Mt == 1))
                    pv = pctx[:, 0:512].rearrange("p (a c) -> p a c", a=2)[:, :, 0:N]
                    pvn = prc[:, 0:512].rearrange("p (a c) -> p a c", a=2)[:, :, 0:N]
                    if off == 0:
                        nc.vector.tensor_mul(
                            ctx_s[qt][0:64, 0:T2].rearrange("p (a c) -> p a c", a=2),
                            pv, pvn)
                    else:
                        nc.vector.tensor_mul(
                            stage[:, 0:T2].rearrange("p (a c) -> p a c", a=2), pv, pvn)
                        nc.sync.dma_start(ctx_s[qt][64:128, :], stage[:])

                # ---- proj / cov_proj + gamma1-scaled residual ----
                for path, w_d in ((0, projw_d), (1, cprojw_d)):
                    pwt = []
                    for kt in range(KT):
                        w = wchunk()
                        nc.sync.dma_start(w[:], w_d[li, 128 * kt:128 * (kt + 1), :])
                        pwt.append(w)
                    win = 0 if path == 0 else COVW
                    vo = 0 if path == 0 else COVO
                    for mt in range(KT):
                        pj = bank([128, NP256])
                        for kt in range(KT):
                            nc.tensor.matmul(pj[:], pwt[kt][:, 128 * mt:128 * (mt + 1)],
                                             ctx_s[kt][:, win:win + NP256],
                                             start=(kt == 0), stop=False)
                        nc.tensor.matmul(pj[:], pbr[:, path * E + 128 * mt:
                                                    path * E + 128 * (mt + 1)],
                                         obrow[:, 0:NP256], start=False, stop=True)
                        nc.vector.scalar_tensor_tensor(
                            xs[mt][:, path * N:(path + 1) * N], pj[:, vo:vo + N],
                            ac(mt, 4), xs[mt][:, path * N:(path + 1) * N],
                            ALU.mult, ALU.add)

                # ---- LN2 + MLP (chunked fc2 accumulation) ----
                layer_norm(xs)
                pf2 = [ps.tile([128, T2], F32, tag="bank", name=f"pf2_{_i}") for _i in range(KT)]
                f1w = {}

                def load_f1(jb):
                    for kt in range(KT):
                        w = wchunk()
                        nc.sync.dma_start(w[:], fc1w_d[li, 128 * kt:128 * (kt + 1),
                                                       E * jb:E * (jb + 1)])
                        f1w[(jb, kt)] = w

                load_f1(0)
                for j in range(MT_H):
                    jb, jm = j // KT, j % KT
                    if jm == 0 and jb + 1 < 4:
                        load_f1(jb + 1)
                    ph = bank([128, T2])
                    for kt in range(KT):
                        nc.tensor.matmul(ph[:], f1w[(jb, kt)][:, 128 * jm:128 * (jm + 1)],
                                         xn[kt][:], start=(kt == 0), stop=(kt == KT - 1))
                    Hj = bp.tile([128, T2], BF, tag="hj")
                    nc.scalar.activation(Hj[:], ph[:], AF.Gelu, bias=f1b[:, j:j + 1])
                    w2 = wchunk()
                    nc.sync.dma_start(w2[:], fc2w_d[li, 128 * j:128 * (j + 1), :])
                    for i in range(KT):
                        nc.tensor.matmul(pf2[i][:], w2[:, 128 * i:128 * (i + 1)], Hj[:],
                                         start=(j == 0), stop=False,
                                         skip_group_check=True)
                for i in range(KT):
                    nc.tensor.matmul(pf2[i][:], pbr[:, 2 * E + 128 * i:2 * E + 128 * (i + 1)],
                                     obrow[:, 0:T2], start=False, stop=True,
                                     skip_group_check=True)
                    nc.vector.scalar_tensor_tensor(xs[i][:], pf2[i][:], ac(i, 5), xs[i][:],
                                                   ALU.mult, ALU.add)

                if debug:
                    for t in range(KT):
                        nc.sync.dma_start(dbg_d[li + 1, 128 * t:128 * (t + 1), :], xs[t][:])

            # ================= head =================
            pl = [rp.tile([128, 2], F32R, tag=f"pool{t}", name=f"pool{t}") for t in range(KT)]
            for t in range(KT):
                nc.vector.tensor_reduce(pl[t][:, 0:1], xs[t][:, 1:N], mybir.AxisListType.X,
                                        ALU.add)
                nc.vector.tensor_scalar(pl[t][:, 0:1], pl[t][:, 0:1], 1.0 / NPATCH,
                                        None, ALU.mult)
                nc.scalar.activation(pl[t][:, 1:2], pl[t][:, 0:1], AF.Square)
            p_s = bank([1, 2])
            for t in range(KT):
                nc.tensor.matmul(p_s[:], onescol, pl[t][:],
                                 start=(t == 0), stop=(t == KT - 1))
            mu = rp.tile([1, 2], F32, tag="hmu")
            nc.vector.tensor_scalar(mu[:], p_s[:], 1.0 / E, None, ALU.mult)
            musq = rp.tile([1, 1], F32, tag="hmusq")
            nc.scalar.activation(musq[:], mu[:, 0:1], AF.Square)
            var = rp.tile([1, 1], F32, tag="hvar")
            nc.vector.tensor_sub(var[:], mu[:, 1:2], musq[:])
            lnv = rp.tile([1, 1], F32, tag="hlnv")
            nc.scalar.activation(lnv[:], var[:], AF.Ln, bias=eps_t[:])
            rr = rp.tile([1, 2], F32R, tag="hrr")
            nc.scalar.activation(rr[:, 0:1], lnv[:], AF.Exp, scale=-0.5)
            nc.vector.tensor_mul(rr[:, 1:2], mu[:, 0:1], rr[:, 0:1])
            p_bc = bank([128, 2])
            nc.tensor.matmul(p_bc[:], onesrow[:, 0:128], rr[:], start=True, stop=True)
            tn = [rp.tile([128, 2], BF, tag=f"tn{t}", name=f"tn{t}") for t in range(KT)]
            for t in range(KT):
                tmp = rp.tile([128, 1], F32, tag="htmp")
                nc.vector.tensor_mul(tmp[:], pl[t][:, 0:1], p_bc[:, 0:1])
                nc.vector.tensor_sub(tn[t][:, 0:1], tmp[:], p_bc[:, 1:2])
                nc.vector.tensor_copy(tn[t][:, 1:2], tn[t][:, 0:1])
            hwt = {}
            for blk in range(2):
                for kt in range(KT):
                    w = wchunk()
                    wd = 768 if blk == 0 else 256
                    nc.sync.dma_start(w[:, 0:wd],
                                      headw_d[128 * kt:128 * (kt + 1),
                                              768 * blk:768 * blk + wd])
                    hwt[(blk, kt)] = w
            hb = rp.tile([128, 8], F32, tag="headb")
            nc.sync.dma_start(hb[:], headb_d[:])
            osb = rp.tile([128, 8], F32, tag="osb")
            for mt in range(8):
                blk, mo = (0, mt) if mt < 6 else (1, mt - 6)
                po = bank([128, 2])
                for kt in range(KT):
                    nc.tensor.matmul(po[:], hwt[(blk, kt)][:, 128 * mo:128 * (mo + 1)],
                                     tn[kt][:], start=(kt == 0), stop=(kt == KT - 1))
                nc.scalar.activation(osb[:, mt:mt + 1], po[:, 0:1], AF.Identity,
                                     bias=hb[:, mt:mt + 1])
            nc.sync.dma_start(out_d[:].rearrange("(a p) c -> p a c", p=128),
                              osb[:].rearrange("p (a c) -> p a c", c=1))

    lp.__exit__(None, None, None)
    nc.finalize()
    return nc


# --------------------------------------------------------------------------
# host-side input preparation
# --------------------------------------------------------------------------

def prep_shared(i):
    """Build the shared (weights etc.) input map from the full input dict."""
    import ml_dtypes
    f = np.float32
    bf = ml_dtypes.bfloat16

    def g(k):
        return np.asarray(i[k], dtype=f)

    # LN affines folded into the consuming weights: qkv <- norm1, fc1 <- norm2,
    # head <- fc_norm. The kernel's layer_norm emits (x-mu)*rstd only.
    g1 = g("norm1_g")          # [L, E]
    b1 = g("norm1_b")
    g2 = g("norm2_g")
    b2 = g("norm2_b")
    qkv_w = g("qkv_w")         # [L, 3E, E]
    qkvw_f = qkv_w * g1[:, None, :]
    badd = np.einsum('loe,le->lo', qkv_w, b1)      # [L, 3E]
    fc1_w = g("fc1_w")         # [L, MLP, E]
    fc1w_f = fc1_w * g2[:, None, :]
    f1badd = np.einsum('lme,le->lm', fc1_w, b2)    # [L, MLP]
    head_w = g("head_w")       # [NCLS, E]
    headw_f = head_w * g("fc_norm_g")[None, :]
    hbadd = head_w @ g("fc_norm_b")                # [NCLS]

    qkvw = np.ascontiguousarray(np.transpose(qkvw_f, (0, 2, 1))).astype(bf)
    projw = np.ascontiguousarray(np.transpose(g("proj_w"), (0, 2, 1))).astype(bf)
    cprojw = np.ascontiguousarray(np.transpose(g("cov_proj_w"), (0, 2, 1))).astype(bf)
    fc1w = np.ascontiguousarray(np.transpose(fc1w_f, (0, 2, 1))).astype(bf)
    fc2w = np.ascontiguousarray(np.transpose(g("fc2_w"), (0, 2, 1))).astype(bf)
    pw = np.ascontiguousarray(g("patch_w").reshape(E, E).T).astype(bf)
    cpw = np.ascontiguousarray(g("cov_patch_w").reshape(E, E).T).astype(bf)
    headw = np.zeros((E, 1024), bf)
    headw[:, 0:NCLS] = headw_f.T.astype(bf)
    rpbT = np.zeros((H, N, NP256), bf)   # holds 2*rpb (tanh/exp half-angle form)
    rpbT[:, :, 0:N] = (2.0 * np.transpose(g("rel_pos_bias"), (0, 2, 1))).astype(bf)

    acols = np.zeros((L, 128, 36), f)
    for c, k in enumerate(["norm1_g", "norm1_b", "norm2_g", "norm2_b",
                           "gamma1", "gamma2"]):
        v = g(k).reshape(L, KT, 128)
        for t in range(KT):
            acols[:, :, 6 * t + c] = v[:, t, :]
    # qkv psum bias columns, with the folded LN1 bias contribution.
    # mean cols: q tiles get 2*SCALE*(q_bias+badd_q); k tiles get badd_k.
    # cov cols: q tiles cov_q_bias+badd_q; k tiles badd_k.
    bq = np.zeros((L, 128, 24), f)
    qb2 = (2.0 * SCALE) * (g("q_bias") + badd[:, 0:E])
    cqb = g("cov_q_bias") + badd[:, 0:E]
    kb = badd[:, E:2 * E]
    for mt in range(KT):
        bq[:, :, 2 * mt] = qb2[:, 128 * mt:128 * (mt + 1)]
        bq[:, :, 2 * mt + 1] = cqb[:, 128 * mt:128 * (mt + 1)]
        bq[:, :, 2 * (KT + mt)] = kb[:, 128 * mt:128 * (mt + 1)]
        bq[:, :, 2 * (KT + mt) + 1] = kb[:, 128 * mt:128 * (mt + 1)]
    fc1b = np.ascontiguousarray((g("fc1_b") + f1badd)
                                .reshape(L, 24, 128).transpose(0, 2, 1))
    vb = np.concatenate([g("v_bias") + badd[:, 2 * E:],
                         g("cov_v_bias") + badd[:, 2 * E:]],
                        axis=1)[:, None, :].astype(bf)
    pbrow = np.concatenate([g("proj_b"), g("cov_proj_b"), g("fc2_b")],
                           axis=1)[:, None, :].astype(bf)
    maskneg = np.zeros((E, H), bf)
    for h in range(H):
        maskneg[64 * h:64 * (h + 1), h] = -1.0
    clspb = np.zeros((E, 4), f)
    clspb[:, 0] = g("cls_tok").reshape(E)
    clspb[:, 1] = g("cov_cls_tok").reshape(E)
    clspb[:, 2] = g("patch_b")
    clspb[:, 3] = g("cov_patch_b")
    hbp = np.zeros(1024, f)
    hbp[0:NCLS] = g("head_b") + hbadd
    headb = np.ascontiguousarray(hbp.reshape(8, 128).T)
    return {
        "qkvw": qkvw, "projw": projw, "cprojw": cprojw, "fc1w": fc1w,
        "fc2w": fc2w, "pw": pw, "cpw": cpw, "headw": headw, "rpbT": rpbT,
        "acols": acols, "bq": bq, "fc1b": fc1b, "vb": vb, "pbrow": pbrow,
        "ones": np.ones((128, 520), f), "onesb": np.ones((128, 520), bf),
        "zeros": np.zeros((1, H * N + 64), bf),
        "maskneg": maskneg, "clspb": clspb,
        "headb": headb,
    }


def im2col(x):
    """x: [B,3,224,224] -> [B, 768, 256] (zero-padded cols, bf16)."""
    import ml_dtypes
    xc = np.asarray(x, dtype=np.float32).reshape(B, 3, 14, 16, 14, 16)
    xc = xc.transpose(0, 1, 3, 5, 2, 4).reshape(B, E, NPATCH)
    out = np.zeros((B, E, NP256), ml_dtypes.bfloat16)
    out[:, :, 0:NPATCH] = xc.astype(ml_dtypes.bfloat16)
    return out


def _get_nc(debug=False, repeats=1):
    key = ("nc", debug, repeats)
    if key not in _CACHE:
        _CACHE[key] = build_nc(debug=debug, repeats=repeats)
    return _CACHE[key]


def run(inputs, debug=False, trace=False, repeats=1, tmpdir=None):
    nc = _get_nc(debug=debug, repeats=repeats)
    shared = prep_shared(inputs)
    xcols = im2col(inputs["x"])
    in_maps = [dict(shared, xcol=np.ascontiguousarray(xcols[b])) for b in range(B)]
    res = run_bass_kernel_spmd(nc, in_maps, list(range(B)), trace=trace,
                               tmpdir=tmpdir)
    y = np.stack([res.results[b]["out"][0:NCLS, 0] for b in range(B)], axis=0)
    return y.astype(np.float32), res


def kernel(**inputs) -> np.ndarray:
    y, _ = run(inputs)
    return y



# revision 58
# speedup vs baseline: 1.3636x; 1.2053x over previous
"""DistVisionTransformer (STOSA-style ViT, mean+cov paths) on 8 Trainium2
NeuronCores. Data-parallel: one image per core; full forward pass on-device
in float32r (TF32-like) matmuls with fp32 accumulation.

Layout: activations are feature-major [768, 394] where columns 0:197 are the
mean-path tokens and 197:394 the cov-path tokens (cls token at cols 0 / 197).
LayerNorm / softmax partition-axis reductions are done with ones-vector
matmuls on the PE; per-token broadcasts with rank-1 ones outer products.
"""
import numpy as np
from contextlib import ExitStack

import concourse.bass as bass
import concourse.bacc as bacc
import concourse.tile as tile
import concourse.mybir as mybir
from concourse.bass_utils import run_bass_kernel_spmd
from concourse.masks import make_identity

F32 = mybir.dt.float32
F32R = mybir.dt.float32r
BF = mybir.dt.bfloat16
AF = mybir.ActivationFunctionType
ALU = mybir.AluOpType

B, E, H, L, P, IMG, NCLS = 8, 768, 12, 12, 16, 224, 1000
D = E // H                  # 64
MLP = 4 * E                 # 3072
SCALE = D ** -0.5
NPATCH = (IMG // P) ** 2    # 196
N = NPATCH + 1              # 197
T2 = 2 * N                  # 394  (mean | cov concatenated along tokens)
NP256 = 256                 # padded token free-dim for 256-wide matmuls
KT = E // 128               # 6 k-tiles over features
MT_H = MLP // 128           # 24 hidden tiles
COVW = T2 - NP256           # 138: start of the 256-wide cov window
COVO = N - COVW             # 59: offset of cov data inside that window

_CACHE = {}


# --------------------------------------------------------------------------
# device kernel builder
# --------------------------------------------------------------------------

def build_nc(debug=False, repeats=1):
    nc = bacc.Bacc(None, target_bir_lowering=False)
    lp = nc.allow_low_precision("tf32-style kernel; fp32 psum accumulation")
    lp.__enter__()

    dp = nc.declare_dram_parameter
    xcol_d = dp("xcol", [E, NP256], BF, isOutput=False)          # per-core im2col
    qkvw_d = dp("qkvw", [L, E, 3 * E], BF, isOutput=False)       # [in, out] (q|k|v)
    projw_d = dp("projw", [L, E, E], BF, isOutput=False)
    cprojw_d = dp("cprojw", [L, E, E], BF, isOutput=False)
    fc1w_d = dp("fc1w", [L, E, MLP], BF, isOutput=False)
    fc2w_d = dp("fc2w", [L, MLP, E], BF, isOutput=False)
    pw_d = dp("pw", [E, E], BF, isOutput=False)                  # patch embed [in, out]
    cpw_d = dp("cpw", [E, E], BF, isOutput=False)
    headw_d = dp("headw", [E, 1024], BF, isOutput=False)         # [in, out] padded
    rpbT_d = dp("rpbT", [H, N, NP256], BF, isOutput=False)       # rpb[h].T, padded
    acols_d = dp("acols", [L, 128, 36], F32, isOutput=False)     # per-tile param cols
    bq_d = dp("bq", [L, 128, 24], F32, isOutput=False)           # qkv psum bias cols
    fc1b_d = dp("fc1b", [L, 128, 24], F32, isOutput=False)
    vb_d = dp("vb", [L, 1, 2 * E], BF, isOutput=False)           # v / cov_v bias rows
    pbrow_d = dp("pbrow", [L, 1, 3 * E], BF, isOutput=False)     # proj|cproj|fc2 bias rows
    ones_d = dp("ones", [128, 520], F32R, isOutput=False)        # all-ones block
    onesb_d = dp("onesb", [128, 520], BF, isOutput=False)        # all-ones block bf16
    zeros_d = dp("zeros", [1, H * N + 64], BF, isOutput=False)
    maskneg_d = dp("maskneg", [E, H], BF, isOutput=False)        # -1 blockdiag
    clspb_d = dp("clspb", [E, 4], F32R, isOutput=False)          # cls|cov_cls|patch_b|cov_patch_b
    headb_d = dp("headb", [128, 8], F32, isOutput=False)
    out_d = dp("out", [1024, 1], F32, isOutput=True)
    if debug:
        dbg_d = dp("dbg", [L + 1, E, T2], F32R, isOutput=True)

    with tile.TileContext(nc) as tc, ExitStack() as ctx:
        pp = ctx.enter_context(tc.tile_pool(name="persist", bufs=1))
        wp = ctx.enter_context(tc.tile_pool(name="weights", bufs=14))
        rp = ctx.enter_context(tc.tile_pool(name="rows", bufs=1))
        bp = ctx.enter_context(tc.tile_pool(name="bigscratch", bufs=2))
        ap = ctx.enter_context(tc.tile_pool(name="attn", bufs=2))
        ps = ctx.enter_context(tc.tile_pool(name="psum", bufs=8, space="PSUM"))

        _bn = [0]

        def bank(shape):
            _bn[0] += 1
            return ps.tile(shape, F32, tag="bank", name=f"pb{_bn[0]}")

        _wn = [0]

        def wchunk():
            _wn[0] += 1
            return wp.tile([128, E], BF, tag="wchunk", name=f"w{_wn[0]}")

        # ---- persistent constants ----
        ones = pp.tile([128, 520], F32R)       # columns / rows of ones (f32r)
        nc.sync.dma_start(ones[:], ones_d[:])
        onescol = ones[:, 0:1]
        onesrow = ones[0:1, :]
        onesb = pp.tile([128, 520], BF)        # bf16 ones
        nc.sync.dma_start(onesb[:], onesb_d[:])
        obcol = onesb[:, 0:1]
        obrow = onesb[0:1, :]
        mask = [pp.tile([128, H], BF, tag=f"mask{t}", name=f"mask{t}") for t in range(KT)]
        for t in range(KT):
            nc.sync.dma_start(mask[t][:], maskneg_d[128 * t:128 * (t + 1), :])
        clspb = [pp.tile([128, 4], F32R, tag=f"clspb{t}", name=f"clspb{t}") for t in range(KT)]
        for t in range(KT):
            nc.sync.dma_start(clspb[t][:], clspb_d[128 * t:128 * (t + 1), :])
        ident = pp.tile([128, 128], F32)
        make_identity(nc, ident[:])
        # resident rel-pos-bias (bf16), loaded once and reused every layer
        rpb_s = [(pp.tile([128, NP256], BF, tag=f"rpba{h}", name=f"rpba{h}"),
                  pp.tile([N - 128, NP256], BF, tag=f"rpbb{h}", name=f"rpbb{h}"))
                 for h in range(H)]
        for h in range(H):
            nc.sync.dma_start(rpb_s[h][0][:], rpbT_d[h, 0:128, :])
            nc.sync.dma_start(rpb_s[h][1][:], rpbT_d[h, 128:N, :])

        # ---- persistent state / per-layer reused buffers ----
        xs = [pp.tile([128, T2], F32R, tag=f"xs{t}", name=f"xs{t}") for t in range(KT)]
        xn = [pp.tile([128, T2], BF, tag=f"xn{t}", name=f"xn{t}") for t in range(KT)]
        xsq = [pp.tile([128, T2], F32R, tag=f"xsq{t}", name=f"xsq{t}") for t in range(KT)]
        qsq = [pp.tile([128, T2], BF, tag=f"qsq{t}", name=f"qsq{t}") for t in range(2 * KT)]
        qkvs = [pp.tile([128, T2], BF, tag=f"qkvs{t}", name=f"qkvs{t}") for t in range(2 * KT)]
        sqk = [pp.tile([128, NP256], BF, tag=f"sqk{t}", name=f"sqk{t}") for t in range(2 * KT)]
        # v token-major with a ones column per 65-wide head block: the ctx
        # matmul's partition 64 then accumulates the softmax denominator.
        VW = 65
        vtm = {}
        for path in (0, 1):
            vtm[path] = (pp.tile([128, H * VW], BF, tag=f"vtm{path}a", name=f"vtm{path}a"),
                         pp.tile([N - 128, H * VW], BF, tag=f"vtm{path}b", name=f"vtm{path}b"))
        ctx_s = [pp.tile([128, T2], BF, tag=f"ctx{t}", name=f"ctx{t}") for t in range(KT)]
        gath = pp.tile([1, H * N + 64], BF)
        nc.sync.dma_start(gath[:], zeros_d[:])   # zero the pad columns once
        an_t = pp.tile([H, N], BF)
        bn_t = pp.tile([H, N], F32)
        bT = (pp.tile([128, H], F32, tag="bT0", name="bT0"), pp.tile([N - 128, H], F32, tag="bT1", name="bT1"))
        stage = pp.tile([64, T2], BF)
        eps_t = pp.tile([1, 1], F32)
        nc.vector.memset(eps_t[:], 1e-5)
        # ones columns of the 65-wide v head blocks (persist across layers;
        # per-layer writes only touch the 0:64 sub-columns)
        for path in (0, 1):
            for Mt in (0, 1):
                mw = (128, N - 128)[Mt]
                dst = vtm[path][Mt][0:mw, :].rearrange(
                    "p (h c) -> p h c", c=VW)[:, :, 64:65]
                nc.vector.tensor_copy(
                    dst, onesb[0:mw, 0:H].rearrange("p (h c) -> p h c", c=1))

        MTOK = (128, N - 128)      # token m-tile sizes
        eps = 1e-5

        U32 = mybir.dt.uint32

        def emit_rsqrt(dst, src, width, tag):
            """dst = 1/sqrt(src), entirely on DVE (quake seed + 2 Newton).
            No ACT table needed. src values must be normal positive f32.
            dst may be f32r: the final mul writes/rounds via its own dtype."""
            # seed bits = M - (i>>1) computed as ~((i>>1) + ~M): the inner sum
            # stays < 2^32 for src < 1e9, safe whether the ALU wraps or
            # saturates.
            ti = rp.tile([1, width], U32, tag=f"{tag}_i")
            nc.vector.tensor_scalar(ti[:], src.bitcast(U32), 1, None,
                                    ALU.logical_shift_right)
            nc.vector.tensor_scalar(ti[:], ti[:], 0xA0C8A61F, None, ALU.add)
            nc.vector.tensor_scalar(ti[:], ti[:], 0xFFFFFFFF, None,
                                    ALU.bitwise_xor)
            y = ti[:].bitcast(F32)
            t1 = rp.tile([1, width], F32, tag=f"{tag}_t")
            y2 = rp.tile([1, width], F32, tag=f"{tag}_y")
            for it in range(2):
                nc.vector.tensor_mul(t1[:], y, y)
                nc.vector.tensor_mul(t1[:], t1[:], src)
                nc.vector.tensor_scalar(t1[:], t1[:], -0.5, 1.5, ALU.mult, ALU.add)
                out = dst if it == 1 else y2[:]
                nc.vector.tensor_mul(out, y, t1[:])
                y = out

        def emit_recip(dst, src, width, tag):
            """dst = 1/src on DVE (bit seed + 2 Newton); src normal positive."""
            # seed bits = M - i as ~(i + ~M); overflow-free for src in [1, 1e9]
            ti = rp.tile([1, width], U32, tag=f"{tag}_i")
            nc.vector.tensor_scalar(ti[:], src.bitcast(U32), 0x810CEE3C, None,
                                    ALU.add)
            nc.vector.tensor_scalar(ti[:], ti[:], 0xFFFFFFFF, None,
                                    ALU.bitwise_xor)
            y = ti[:].bitcast(F32)
            t1 = rp.tile([1, width], F32, tag=f"{tag}_t")
            for _ in range(2):
                nc.vector.tensor_mul(t1[:], y, src)
                nc.vector.tensor_scalar(t1[:], t1[:], -1.0, 2.0, ALU.mult, ALU.add)
                nc.vector.tensor_mul(dst, y, t1[:])
                y = dst

        def layer_norm(src):
            """src: 6 [128,T2] f32r tiles -> xn bf16 ((x-mu)*rstd; affine folded
            into the consuming weights host-side). All-DVE rstd: no ACT ops."""
            for t in range(KT):
                nc.scalar.activation(xsq[t][:], src[t][:], AF.Square)
            p_s = bank([1, T2])
            p_s2 = bank([1, T2])
            for t in range(KT):
                nc.tensor.matmul(p_s[:], onescol, src[t][:],
                                 start=(t == 0), stop=(t == KT - 1))
            for t in range(KT):
                nc.tensor.matmul(p_s2[:], onescol, xsq[t][:],
                                 start=(t == 0), stop=(t == KT - 1))
            mu = rp.tile([1, T2], F32, tag="ln_mu")
            nc.vector.tensor_scalar(mu[:], p_s[:], 1.0 / E, None, ALU.mult)
            var = rp.tile([1, T2], F32, tag="ln_var")
            nc.vector.tensor_scalar(var[:], p_s2[:], 1.0 / E, eps,
                                    ALU.mult, ALU.add)
            musq = rp.tile([1, T2], F32, tag="ln_musq")
            nc.vector.tensor_mul(musq[:], mu[:], mu[:])
            nc.vector.tensor_sub(var[:], var[:], musq[:])
            rinv_r = rp.tile([1, T2], F32R, tag="ln_rinvr")
            emit_rsqrt(rinv_r[:], var[:], T2, "lnr")
            mur = rp.tile([1, T2], F32R, tag="ln_mur")
            nc.vector.tensor_mul(mur[:], mu[:], rinv_r[:])
            p_R = bank([128, T2])
            p_MR = bank([128, T2])
            nc.tensor.matmul(p_R[:], onesrow[:, 0:128], rinv_r[:], start=True, stop=True)
            nc.tensor.matmul(p_MR[:], onesrow[:, 0:128], mur[:], start=True, stop=True)
            for t in range(KT):
                tmp = bp.tile([128, T2], F32, tag="ln_tmp")
                nc.vector.tensor_mul(tmp[:], src[t][:], p_R[:])
                nc.vector.tensor_sub(xn[t][:], tmp[:], p_MR[:])

        for _rep in range(repeats):
            # ================= patch embed =================
            xcol = [pp.tile([128, NP256], BF, tag=f"xcol{t}", name=f"xcol{t}") for t in range(KT)]
            for t in range(KT):
                nc.sync.dma_start(xcol[t][:], xcol_d[128 * t:128 * (t + 1), :])

            for path, w_d in ((0, pw_d), (1, cpw_d)):
                wt = []
                for kt in range(KT):
                    w = wchunk()
                    nc.sync.dma_start(w[:], w_d[128 * kt:128 * (kt + 1), :])
                    wt.append(w)
                for mt in range(KT):
                    pe = bank([128, NP256])
                    for kt in range(KT):
                        nc.tensor.matmul(pe[:], wt[kt][:, 128 * mt:128 * (mt + 1)],
                                         xcol[kt][:], start=(kt == 0), stop=(kt == KT - 1))
                    dst = xs[mt][:, 1 + path * N: 1 + path * N + NPATCH]
                    nc.scalar.activation(dst, pe[:, 0:NPATCH], AF.Identity,
                                         bias=clspb[mt][:, 2 + path: 3 + path], scale=1.0)
            # cls tokens into cols 0 / 197
            for t in range(KT):
                nc.vector.tensor_copy(
                    xs[t][:, 0:T2].rearrange("p (a c) -> p a c", a=2)[:, :, 0:1],
                    clspb[t][:, 0:2].rearrange("p (a c) -> p a c", a=2)[:, :, 0:1])

            if debug:
                for t in range(KT):
                    nc.sync.dma_start(dbg_d[0, 128 * t:128 * (t + 1), :], xs[t][:])

            # ================= transformer layers =================
            for li in range(L):
                A = rp.tile([128, 36], F32, tag="acols")
                nc.sync.dma_start(A[:], acols_d[li])
                Bq = rp.tile([128, 24], F32, tag="bqcols")
                nc.sync.dma_start(Bq[:], bq_d[li])
                f1b = rp.tile([128, 24], F32, tag="fc1b")
                nc.sync.dma_start(f1b[:], fc1b_d[li])
                vb = rp.tile([1, 2 * E], BF, tag="vbrow")
                nc.sync.dma_start(vb[:], vb_d[li])
                pbr = rp.tile([1, 3 * E], BF, tag="pbrow")
                nc.sync.dma_start(pbr[:], pbrow_d[li])

                def ac(t, c):
                    return A[:, 6 * t + c: 6 * t + c + 1]

                # ---- LN1 (affine folded into qkv weights host-side) ----
                layer_norm(xs)

                # ---- qkv (q|k part), feature-major, both paths at once ----
                # post-processing fused per m-tile so ACT/DVE overlap the
                # remaining matmuls; sqrt done as exp(0.5*ln(.)) to stay in
                # the natural_log_exp table set (no sqrt-set thrash).
                LN2C = float(np.log(2.0))
                qkw = {}
                for b in (0, 1):
                    for kt in range(KT):
                        w = wchunk()
                        nc.sync.dma_start(w[:], qkvw_d[li, 128 * kt:128 * (kt + 1),
                                                       E * b:E * (b + 1)])
                        qkw[(b, kt)] = w
                for mt in range(2 * KT):
                    b, m = mt // KT, mt % KT
                    pq = bank([128, T2])
                    for kt in range(KT):
                        nc.tensor.matmul(pq[:], qkw[(b, kt)][:, 128 * m:128 * (m + 1)],
                                         xn[kt][:], start=(kt == 0), stop=(kt == KT - 1))
                    # mean columns (bias cols already hold folded LN biases)
                    if mt < KT:   # q rows: * 2*SCALE, + bias
                        nc.vector.tensor_scalar(qkvs[mt][:, 0:N], pq[:, 0:N],
                                                2.0 * SCALE, Bq[:, 2 * mt:2 * mt + 1],
                                                ALU.mult, ALU.add)
                        # a-term source: (q_scaled)^2 (full scale)
                        nc.scalar.activation(qsq[mt][:, 0:N], qkvs[mt][:, 0:N],
                                             AF.Square, scale=0.5)
                    else:         # k rows: + bias; b-term source: 0.5*k^2
                        nc.vector.tensor_scalar(qkvs[mt][:, 0:N], pq[:, 0:N],
                                                1.0, Bq[:, 2 * mt:2 * mt + 1],
                                                ALU.mult, ALU.add)
                        nc.scalar.activation(qsq[mt][:, 0:N], qkvs[mt][:, 0:N],
                                             AF.Square, scale=float(np.sqrt(0.5)))
                    # cov columns: elu(x + b) + 1 = exp(min(x+b,0)) + max(x+b,0)
                    e1 = bp.tile([128, N], F32, tag="elu1")
                    e2 = bp.tile([128, N], F32, tag="elu2")
                    bcol = Bq[:, 2 * mt + 1:2 * mt + 2]
                    nc.vector.tensor_scalar(e1[:], pq[:, N:T2], bcol, 0.0, ALU.add, ALU.min)
                    nc.scalar.activation(e1[:], e1[:], AF.Exp)
                    nc.vector.tensor_scalar(e2[:], pq[:, N:T2], bcol, 0.0, ALU.add, ALU.max)
                    nc.vector.tensor_add(qkvs[mt][:, N:T2], e1[:], e2[:])
                    # a/b cov source (k half-weighted for the tanh bias)
                    if mt < KT:
                        nc.vector.tensor_copy(qsq[mt][:, N:T2], qkvs[mt][:, N:T2])
                    else:
                        nc.vector.tensor_scalar(qsq[mt][:, N:T2], qkvs[mt][:, N:T2],
                                                0.5, None, ALU.mult)

                # sq = sqrt(4*cq); sk = sqrt(ck). The zero-valued gate column
                # (computed from the last tile's cov output) forces every Sqrt
                # after the last elu Exp, so the ACT table switches exactly
                # once per phase instead of thrashing per tile. Tiles are
                # ordered (k0,q0,k1,q1,...) so head pairs unblock early.
                gate = rp.tile([128, 1], F32, tag="gate")
                nc.vector.tensor_scalar(gate[:], qkvs[2 * KT - 1][:, T2 - 1:T2],
                                        0.0, None, ALU.mult)
                for m in range(KT):
                    for mt in (KT + m, m):
                        nc.scalar.activation(sqk[mt][:, 0:N], qkvs[mt][:, N:T2],
                                             AF.Sqrt, bias=gate[:],
                                             scale=4.0 if mt < KT else 1.0)

                # ---- a/b norm terms via -1-blockdiag mask matmuls ----
                p_a = bank([H, T2])
                p_b = bank([H, T2])
                for half, pdst in ((0, p_a), (1, p_b)):
                    for kt in range(KT):
                        nc.tensor.matmul(pdst[:], mask[kt][:], qsq[half * KT + kt][:],
                                         start=(kt == 0), stop=(kt == KT - 1))
                nc.vector.tensor_copy(an_t[:], p_a[:, 0:N])
                nc.vector.tensor_add(an_t[:], an_t[:], p_a[:, N:T2])
                nc.vector.tensor_copy(bn_t[:], p_b[:, 0:N])
                nc.vector.tensor_add(bn_t[:], bn_t[:], p_b[:, N:T2])
                # gather -a rows to partition 0 (strided per-head 256 slots)
                nc.sync.dma_start(
                    gath[:, 0:H * N].rearrange("p (h c) -> p h c", c=N), an_t[:])
                # transpose -b/2 to per-token columns (tanh bias)
                for Mt in range(2):
                    moff, mw = 128 * Mt, MTOK[Mt]
                    pt = bank([mw, H])
                    nc.tensor.transpose(pt[:], bn_t[:, moff:moff + mw], ident[0:H, 0:H])
                    nc.vector.tensor_copy(bT[Mt][:], pt[:])

                # ---- v token-major (both paths), weight as moving operand ----
                for kt in range(KT):
                    w = wchunk()
                    nc.sync.dma_start(w[:], qkvw_d[li, 128 * kt:128 * (kt + 1), 2 * E:3 * E])
                    qkw[(2, kt)] = w
                for path in (0, 1):
                    for Mt in range(2):
                        moff, mw = 128 * Mt, MTOK[Mt]
                        for ch in range(2):
                            pv = bank([mw, 384])
                            for kt in range(KT):
                                nc.tensor.matmul(
                                    pv[:], xn[kt][:, path * N + moff: path * N + moff + mw],
                                    qkw[(2, kt)][:, 384 * ch:384 * (ch + 1)],
                                    start=(kt == 0), stop=False)
                            nc.tensor.matmul(pv[:], obrow[:, 0:mw],
                                             vb[:, path * E + 384 * ch: path * E + 384 * (ch + 1)],
                                             start=False, stop=True)
                            dst = vtm[path][Mt][0:mw, 6 * VW * ch:6 * VW * (ch + 1)] \
                                .rearrange("p (h c) -> p h c", c=VW)[:, :, 0:64]
                            pvv = pv[:].rearrange("p (h c) -> p h c", c=64)
                            if path == 0:
                                nc.vector.tensor_copy(dst, pvv)
                            else:
                                e1 = bp.tile([128, 384], F32, tag="velu1")
                                e2 = bp.tile([128, 384], F32, tag="velu2")
                                nc.vector.tensor_scalar(e1[0:mw, :], pv[:], 0.0, None, ALU.min)
                                nc.scalar.activation(e1[0:mw, :], e1[0:mw, :], AF.Exp)
                                nc.vector.tensor_scalar(e2[0:mw, :], pv[:], 0.0, None, ALU.max)
                                nc.vector.tensor_add(
                                    dst, e1[0:mw, :].rearrange("p (h c) -> p h c", c=64),
                                    e2[0:mw, :].rearrange("p (h c) -> p h c", c=64))

                # ---- attention, head by head ----
                # sigmoid(s) = 0.5 + 0.5*tanh(s/2): tanh+exp share one ACT
                # table set. Softmax normalization deferred to the ctx output:
                # ctx_mean *= 1/d, ctx_cov *= 1/d^2 (per query), so the
                # reciprocal is off the per-head critical path.
                def finish_head(st):
                    """Deferred normalize+store for a head (runs one head late
                    so the PE never stalls on the reciprocal chain)."""
                    qt, off, pctx, rr2 = st
                    prh = bank([64, 512])
                    nc.tensor.matmul(prh[:], obrow[:, 0:64], rr2[:],
                                     start=True, stop=True)
                    prc = ap.tile([64, 512], BF, tag="prc", bufs=2)
                    nc.scalar.activation(prc[:], prh[:], AF.Copy)
                    pv = pctx[0:64, 0:512].rearrange("p (a c) -> p a c", a=2)[:, :, 0:N]
                    pvn = prc[:, 0:512].rearrange("p (a c) -> p a c", a=2)[:, :, 0:N]
                    if off == 0:
                        nc.vector.tensor_mul(
                            ctx_s[qt][0:64, 0:T2].rearrange("p (a c) -> p a c", a=2),
                            pv, pvn)
                    else:
                        nc.vector.tensor_mul(
                            stage[:, 0:T2].rearrange("p (a c) -> p a c", a=2), pv, pvn)
                        nc.sync.dma_start(ctx_s[qt][64:128, :], stage[:])

                hstate = None
                last_Ee = None
                for h in range(H):
                    qt, off = h // 2, 64 * (h % 2)
                    Et, E2t, psc = [], [], []
                    for Mt in range(2):
                        moff, mw = 128 * Mt, MTOK[Mt]
                        pc = bank([mw, NP256])
                        nc.tensor.matmul(pc[:], qkvs[KT + qt][off:off + 64, moff:moff + mw],
                                         qkvs[qt][off:off + 64, 0:NP256],
                                         start=True, stop=False)
                        nc.tensor.matmul(pc[:], sqk[KT + qt][off:off + 64, moff:moff + mw],
                                         sqk[qt][off:off + 64, 0:NP256],
                                         start=False, stop=False)
                        nc.tensor.matmul(pc[:], obrow[:, 0:mw],
                                         gath[:, N * h:N * h + NP256],
                                         start=False, stop=True)
                        psc.append(pc)
                    for Mt in range(2):
                        mw = MTOK[Mt]
                        sg = ap.tile([128, NP256], BF, tag="sig", bufs=3)
                        nc.scalar.activation(sg[0:mw, :], psc[Mt][:], AF.Tanh,
                                             bias=bT[Mt][:, h:h + 1], scale=0.5)
                        nc.vector.tensor_add(sg[0:mw, :], sg[0:mw, :], rpb_s[h][Mt][:])
                        Ee = ap.tile([128, NP256], BF, tag="E", bufs=3)
                        # exp(0.5*sg) = exp(sigmoid(s)+rpb)/e^0.5: the constant
                        # factor cancels in the deferred normalization.
                        nc.scalar.activation(Ee[0:mw, :], sg[0:mw, :], AF.Exp,
                                             scale=0.5)
                        Et.append(Ee)
                        E2 = ap.tile([128, NP256], BF, tag="E2", bufs=3)
                        nc.vector.tensor_mul(E2[0:mw, :], Ee[0:mw, :], Ee[0:mw, :])
                        E2t.append(E2)
                    last_Ee = Et[1]
                    # ctx (and, on partition 64, the softmax denominator via
                    # the ones column of vtm)
                    pctx = bank([VW, 512])
                    for path in (0, 1):
                        src = Et if path == 0 else E2t
                        for Mt in range(2):
                            mw = MTOK[Mt]
                            nc.tensor.matmul(pctx[:, 256 * path:256 * (path + 1)],
                                             vtm[path][Mt][:, VW * h:VW * (h + 1)],
                                             src[Mt][0:mw, :],
                                             start=(Mt == 0), stop=(Mt == 1))
                    dcp = ap.tile([1, NP256], F32, tag="dcp", bufs=2)
                    nc.vector.tensor_copy(dcp[:], pctx[64:VW, 0:NP256])
                    rd = ap.tile([1, NP256], F32, tag="rd", bufs=2)
                    emit_recip(rd[:], dcp[:], NP256, "rdq")
                    rr2 = ap.tile([1, 512], BF, tag="rr2", bufs=2)
                    nc.vector.tensor_copy(rr2[:, 0:NP256], rd[:])
                    nc.vector.tensor_mul(rr2[:, NP256:512], rd[:], rd[:])
                    if hstate is not None:
                        finish_head(hstate)
                    hstate = (qt, off, pctx, rr2)
                finish_head(hstate)

                # ---- proj / cov_proj + gamma1-scaled residual ----
                for path, w_d in ((0, projw_d), (1, cprojw_d)):
                    pwt = []
                    for kt in range(KT):
                        w = wchunk()
                        nc.sync.dma_start(w[:], w_d[li, 128 * kt:128 * (kt + 1), :])
                        pwt.append(w)
                    win = 0 if path == 0 else COVW
                    vo = 0 if path == 0 else COVO
                    for mt in range(KT):
                        pj = bank([128, NP256])
                        for kt in range(KT):
                            nc.tensor.matmul(pj[:], pwt[kt][:, 128 * mt:128 * (mt + 1)],
                                             ctx_s[kt][:, win:win + NP256],
                                             start=(kt == 0), stop=False)
                        nc.tensor.matmul(pj[:], pbr[:, path * E + 128 * mt:
                                                    path * E + 128 * (mt + 1)],
                                         obrow[:, 0:NP256], start=False, stop=True)
                        nc.vector.scalar_tensor_tensor(
                            xs[mt][:, path * N:(path + 1) * N], pj[:, vo:vo + N],
                            ac(mt, 4), xs[mt][:, path * N:(path + 1) * N],
                            ALU.mult, ALU.add)

                # ---- LN2 + MLP (chunked fc2 accumulation) ----
                layer_norm(xs)
                pf2 = [ps.tile([128, T2], F32, tag="bank", name=f"pf2_{_i}") for _i in range(KT)]
                f1w = {}

                def load_f1(jb):
                    for kt in range(KT):
                        w = wchunk()
                        nc.sync.dma_start(w[:], fc1w_d[li, 128 * kt:128 * (kt + 1),
                                                       E * jb:E * (jb + 1)])
                        f1w[(jb, kt)] = w

                load_f1(0)

                def fc2_block(st):
                    j, Hj, w2 = st
                    for i in range(KT):
                        nc.tensor.matmul(pf2[i][:], w2[:, 128 * i:128 * (i + 1)], Hj[:],
                                         start=(j == 0), stop=False,
                                         skip_group_check=True)

                mstate = None
                for j in range(MT_H):
                    jb, jm = j // KT, j % KT
                    if jm == 0 and jb + 1 < 4:
                        load_f1(jb + 1)
                    ph = bank([128, T2])
                    for kt in range(KT):
                        nc.tensor.matmul(ph[:], f1w[(jb, kt)][:, 128 * jm:128 * (jm + 1)],
                                         xn[kt][:], start=(kt == 0), stop=(kt == KT - 1))
                    Hj = bp.tile([128, T2], BF, tag="hj", bufs=3)
                    nc.scalar.activation(Hj[:], ph[:], AF.Gelu, bias=f1b[:, j:j + 1])
                    w2 = wchunk()
                    nc.sync.dma_start(w2[:], fc2w_d[li, 128 * j:128 * (j + 1), :])
                    if mstate is not None:
                        fc2_block(mstate)
                    mstate = (j, Hj, w2)
                fc2_block(mstate)
                for i in range(KT):
                    nc.tensor.matmul(pf2[i][:], pbr[:, 2 * E + 128 * i:2 * E + 128 * (i + 1)],
                                     obrow[:, 0:T2], start=False, stop=True,
                                     skip_group_check=True)
                    nc.vector.scalar_tensor_tensor(xs[i][:], pf2[i][:], ac(i, 5), xs[i][:],
                                                   ALU.mult, ALU.add)

                if debug:
                    for t in range(KT):
                        nc.sync.dma_start(dbg_d[li + 1, 128 * t:128 * (t + 1), :], xs[t][:])

            # ================= head =================
            pl = [rp.tile([128, 2], F32R, tag=f"pool{t}", name=f"pool{t}") for t in range(KT)]
            for t in range(KT):
                nc.vector.tensor_reduce(pl[t][:, 0:1], xs[t][:, 1:N], mybir.AxisListType.X,
                                        ALU.add)
                nc.vector.tensor_scalar(pl[t][:, 0:1], pl[t][:, 0:1], 1.0 / NPATCH,
                                        None, ALU.mult)
                nc.scalar.activation(pl[t][:, 1:2], pl[t][:, 0:1], AF.Square)
            p_s = bank([1, 2])
            for t in range(KT):
                nc.tensor.matmul(p_s[:], onescol, pl[t][:],
                                 start=(t == 0), stop=(t == KT - 1))
            mu = rp.tile([1, 2], F32, tag="hmu")
            nc.vector.tensor_scalar(mu[:], p_s[:], 1.0 / E, None, ALU.mult)
            musq = rp.tile([1, 1], F32, tag="hmusq")
            nc.scalar.activation(musq[:], mu[:, 0:1], AF.Square)
            var = rp.tile([1, 1], F32, tag="hvar")
            nc.vector.tensor_sub(var[:], mu[:, 1:2], musq[:])
            nc.vector.tensor_scalar(var[:], var[:], 1.0, eps, ALU.mult, ALU.add)
            rr = rp.tile([1, 2], F32R, tag="hrr")
            emit_rsqrt(rr[:, 0:1], var[:], 1, "hrq")
            nc.vector.tensor_mul(rr[:, 1:2], mu[:, 0:1], rr[:, 0:1])
            p_bc = bank([128, 2])
            nc.tensor.matmul(p_bc[:], onesrow[:, 0:128], rr[:], start=True, stop=True)
            tn = [rp.tile([128, 2], BF, tag=f"tn{t}", name=f"tn{t}") for t in range(KT)]
            for t in range(KT):
                tmp = rp.tile([128, 1], F32, tag="htmp")
                nc.vector.tensor_mul(tmp[:], pl[t][:, 0:1], p_bc[:, 0:1])
                nc.vector.tensor_sub(tn[t][:, 0:1], tmp[:], p_bc[:, 1:2])
                nc.vector.tensor_copy(tn[t][:, 1:2], tn[t][:, 0:1])
            hwt = {}
            for blk in range(2):
                for kt in range(KT):
                    w = wchunk()
                    wd = 768 if blk == 0 else 256
                    nc.sync.dma_start(w[:, 0:wd],
                                      headw_d[128 * kt:128 * (kt + 1),
                                              768 * blk:768 * blk + wd])
                    hwt[(blk, kt)] = w
            hb = rp.tile([128, 8], F32, tag="headb")
            nc.sync.dma_start(hb[:], headb_d[:])
            osb = rp.tile([128, 8], F32, tag="osb")
            for mt in range(8):
                blk, mo = (0, mt) if mt < 6 else (1, mt - 6)
                po = bank([128, 2])
                for kt in range(KT):
                    nc.tensor.matmul(po[:], hwt[(blk, kt)][:, 128 * mo:128 * (mo + 1)],
                                     tn[kt][:], start=(kt == 0), stop=(kt == KT - 1))
                nc.scalar.activation(osb[:, mt:mt + 1], po[:, 0:1], AF.Identity,
                                     bias=hb[:, mt:mt + 1])
            nc.sync.dma_start(out_d[:].rearrange("(a p) c -> p a c", p=128),
                              osb[:].rearrange("p (a c) -> p a c", c=1))

    lp.__exit__(None, None, None)
    nc.finalize()
    return nc


# --------------------------------------------------------------------------
# host-side input preparation
# --------------------------------------------------------------------------

def prep_shared(i):
    """Build the shared (weights etc.) input map from the full input dict."""
    import ml_dtypes
    f = np.float32
    bf = ml_dtypes.bfloat16

    def g(k):
        return np.asarray(i[k], dtype=f)

    # LN affines folded into the consuming weights: qkv <- norm1, fc1 <- norm2,
    # head <- fc_norm. The kernel's layer_norm emits (x-mu)*rstd only.
    g1 = g("norm1_g")          # [L, E]
    b1 = g("norm1_b")
    g2 = g("norm2_g")
    b2 = g("norm2_b")
    qkv_w = g("qkv_w")         # [L, 3E, E]
    qkvw_f = qkv_w * g1[:, None, :]
    badd = np.einsum('loe,le->lo', qkv_w, b1)      # [L, 3E]
    fc1_w = g("fc1_w")         # [L, MLP, E]
    fc1w_f = fc1_w * g2[:, None, :]
    f1badd = np.einsum('lme,le->lm', fc1_w, b2)    # [L, MLP]
    head_w = g("head_w")       # [NCLS, E]
    headw_f = head_w * g("fc_norm_g")[None, :]
    hbadd = head_w @ g("fc_norm_b")                # [NCLS]

    qkvw = np.ascontiguousarray(np.transpose(qkvw_f, (0, 2, 1))).astype(bf)
    projw = np.ascontiguousarray(np.transpose(g("proj_w"), (0, 2, 1))).astype(bf)
    cprojw = np.ascontiguousarray(np.transpose(g("cov_proj_w"), (0, 2, 1))).astype(bf)
    fc1w = np.ascontiguousarray(np.transpose(fc1w_f, (0, 2, 1))).astype(bf)
    fc2w = np.ascontiguousarray(np.transpose(g("fc2_w"), (0, 2, 1))).astype(bf)
    pw = np.ascontiguousarray(g("patch_w").reshape(E, E).T).astype(bf)
    cpw = np.ascontiguousarray(g("cov_patch_w").reshape(E, E).T).astype(bf)
    headw = np.zeros((E, 1024), bf)
    headw[:, 0:NCLS] = headw_f.T.astype(bf)
    rpbT = np.zeros((H, N, NP256), bf)   # holds 2*rpb (tanh/exp half-angle form)
    rpbT[:, :, 0:N] = (2.0 * np.transpose(g("rel_pos_bias"), (0, 2, 1))).astype(bf)

    acols = np.zeros((L, 128, 36), f)
    for c, k in enumerate(["norm1_g", "norm1_b", "norm2_g", "norm2_b",
                           "gamma1", "gamma2"]):
        v = g(k).reshape(L, KT, 128)
        for t in range(KT):
            acols[:, :, 6 * t + c] = v[:, t, :]
    # qkv psum bias columns, with the folded LN1 bias contribution.
    # mean cols: q tiles get 2*SCALE*(q_bias+badd_q); k tiles get badd_k.
    # cov cols: q tiles cov_q_bias+badd_q; k tiles badd_k.
    bq = np.zeros((L, 128, 24), f)
    qb2 = (2.0 * SCALE) * (g("q_bias") + badd[:, 0:E])
    cqb = g("cov_q_bias") + badd[:, 0:E]
    kb = badd[:, E:2 * E]
    for mt in range(KT):
        bq[:, :, 2 * mt] = qb2[:, 128 * mt:128 * (mt + 1)]
        bq[:, :, 2 * mt + 1] = cqb[:, 128 * mt:128 * (mt + 1)]
        bq[:, :, 2 * (KT + mt)] = kb[:, 128 * mt:128 * (mt + 1)]
        bq[:, :, 2 * (KT + mt) + 1] = kb[:, 128 * mt:128 * (mt + 1)]
    fc1b = np.ascontiguousarray((g("fc1_b") + f1badd)
                                .reshape(L, 24, 128).transpose(0, 2, 1))
    vb = np.concatenate([g("v_bias") + badd[:, 2 * E:],
                         g("cov_v_bias") + badd[:, 2 * E:]],
                        axis=1)[:, None, :].astype(bf)
    pbrow = np.concatenate([g("proj_b"), g("cov_proj_b"), g("fc2_b")],
                           axis=1)[:, None, :].astype(bf)
    maskneg = np.zeros((E, H), bf)
    for h in range(H):
        maskneg[64 * h:64 * (h + 1), h] = -1.0
    clspb = np.zeros((E, 4), f)
    clspb[:, 0] = g("cls_tok").reshape(E)
    clspb[:, 1] = g("cov_cls_tok").reshape(E)
    clspb[:, 2] = g("patch_b")
    clspb[:, 3] = g("cov_patch_b")
    hbp = np.zeros(1024, f)
    hbp[0:NCLS] = g("head_b") + hbadd
    headb = np.ascontiguousarray(hbp.reshape(8, 128).T)
    return {
        "qkvw": qkvw, "projw": projw, "cprojw": cprojw, "fc1w": fc1w,
        "fc2w": fc2w, "pw": pw, "cpw": cpw, "headw": headw, "rpbT": rpbT,
        "acols": acols, "bq": bq, "fc1b": fc1b, "vb": vb, "pbrow": pbrow,
        "ones": np.ones((128, 520), f), "onesb": np.ones((128, 520), bf),
        "zeros": np.zeros((1, H * N + 64), bf),
        "maskneg": maskneg, "clspb": clspb,
        "headb": headb,
    }


def im2col(x):
    """x: [B,3,224,224] -> [B, 768, 256] (zero-padded cols, bf16)."""
    import ml_dtypes
    xc = np.asarray(x, dtype=np.float32).reshape(B, 3, 14, 16, 14, 16)
    xc = xc.transpose(0, 1, 3, 5, 2, 4).reshape(B, E, NPATCH)
    out = np.zeros((B, E, NP256), ml_dtypes.bfloat16)
    out[:, :, 0:NPATCH] = xc.astype(ml_dtypes.bfloat16)
    return out


def _get_nc(debug=False, repeats=1):
    key = ("nc", debug, repeats)
    if key not in _CACHE:
        _CACHE[key] = build_nc(debug=debug, repeats=repeats)
    return _CACHE[key]


def run(inputs, debug=False, trace=False, repeats=1, tmpdir=None):
    nc = _get_nc(debug=debug, repeats=repeats)
    shared = prep_shared(inputs)
    xcols = im2col(inputs["x"])
    in_maps = [dict(shared, xcol=np.ascontiguousarray(xcols[b])) for b in range(B)]
    res = run_bass_kernel_spmd(nc, in_maps, list(range(B)), trace=trace,
                               tmpdir=tmpdir)
    y = np.stack([res.results[b]["out"][0:NCLS, 0] for b in range(B)], axis=0)
    return y.astype(np.float32), res


def kernel(**inputs) -> np.ndarray:
    y, _ = run(inputs)
    return y

